# revision 1
# baseline (speedup 1.0000x reference)
"""Trainium2 Bass kernel for nn_Attention_87857851006980.

Sharding: 8 cores = 4 batches x 2 head-halves. Core c handles batch c//2,
heads [0..4) (even c) or [4..8) (odd c). Each core computes qkv for its
heads (full-d contraction), the conv/spe branches for its heads' channels,
attention for its heads, and a partial nn1 over its 512 channels; a
pair-wise ReduceScatter then sums the nn1 partials, leaving each core with
half of the output channels for its batch. Host gathers/transposes.

All heavy math runs on-device; the host only slices/transposes inputs
(layout) and folds BN/bias constants into weight tensors.
"""
import sys
sys.path.insert(0, "/opt/trn_rl_repo")
import numpy as np

import concourse.bacc as bacc
import concourse.bass as bass
import concourse.tile as tile
import concourse.mybir as mybir
from concourse.bass_utils import run_bass_kernel_spmd

F32 = mybir.dt.float32
F32R = mybir.dt.float32r
BF16 = mybir.dt.bfloat16
AF = mybir.ActivationFunctionType
ADD = mybir.AluOpType.add
MULT = mybir.AluOpType.mult

B, D, N, H, HD = 4, 1024, 2304, 8, 128
PS = 48          # image side; N = PS*PS
PP = PS + 2      # padded side
NPAD = PP * PP + 2  # 2502: +2 so the (dy,dx)=(1,1) view of the last row-chunk stays in bounds
HPC = 4          # heads per core
CH = HPC * HD    # 512 channels per core
EPS = 1e-5
SCALE = D ** -0.5

# n-chunks of the free dimension (all >=256 so f32r runs at full rate)
CHUNKS = [(0, 512), (512, 512), (1024, 512), (1536, 512), (2048, 256)]
NJ = N // 128    # 18 key blocks
EB = 8           # qkv output blocks per core: 4 q + 4 k (v folded into wvsum)

USE_COLLECTIVE = True


def _build(single=False):
    use_cc = USE_COLLECTIVE and not single
    nc = bacc.Bacc("TRN2", target_bir_lowering=False, debug=False,
                   num_devices=1 if single else 8)

    # ---- DRAM I/O ----
    x_dn = nc.dram_tensor("x_dn", [D, N], F32, kind="ExternalInput").ap()
    xpad_d = nc.dram_tensor("xpad", [CH, NPAD], F32, kind="ExternalInput").ap()
    wqkv_d = nc.dram_tensor("wqkv", [128, 8 * EB * 128], F32, kind="ExternalInput").ap()
    wvsum_d = nc.dram_tensor("wvsum", [128, 8 * HPC], F32, kind="ExternalInput").ap()
    bqk_d = nc.dram_tensor("bqk", [128, 8], F32, kind="ExternalInput").ap()
    vbias_d = nc.dram_tensor("vbias", [128, HPC], F32, kind="ExternalInput").ap()
    wconv_d = nc.dram_tensor("wconv", [128, HPC * 9 * 128], F32, kind="ExternalInput").ap()
    wspe_d = nc.dram_tensor("wspe", [128, HPC * 128], F32, kind="ExternalInput").ap()
    bn_s_d = nc.dram_tensor("bn_s", [128, HPC], F32, kind="ExternalInput").ap()
    bn_b_d = nc.dram_tensor("bn_b", [128, HPC], F32, kind="ExternalInput").ap()
    bnc_s_d = nc.dram_tensor("bnc_s", [128, HPC], F32, kind="ExternalInput").ap()
    bnc_b_d = nc.dram_tensor("bnc_b", [128, HPC], F32, kind="ExternalInput").ap()
    wnn1_d = nc.dram_tensor("wnn1", [128, HPC * D], F32, kind="ExternalInput").ap()
    bnn1_d = nc.dram_tensor("bnn1h", [128, 8], F32, kind="ExternalInput").ap()
    ones_d = nc.dram_tensor("onesc", [128, 2], F32, kind="ExternalInput").ap()
    ident_d = nc.dram_tensor("ident", [128, 128], F32, kind="ExternalInput").ap()
    if use_cc:
        out_d = nc.dram_tensor("out", [CH, N], F32, kind="ExternalOutput").ap()
    else:
        out_d = nc.dram_tensor("out", [D, N], F32, kind="ExternalOutput").ap()

    with tile.TileContext(nc) as tc:
      with tc.tile_pool(name="persist", bufs=1) as pp:
        # ---------- persistent tiles ----------
        qk_sb = pp.tile([128, 8 * N], F32R, tag="qk")   # q then k, 4 heads each
        vpT_sb = pp.tile([128, HPC * NJ * 128], BF16, tag="vpT")  # V' (cbr^T, then +v_spe)
        vcol_all = pp.tile([128, HPC * NJ], F32, tag="vcol_all")
        spe_row = pp.tile([1, HPC * 128], F32R, tag="spe_row")
        ones_sb = pp.tile([128, 2], F32R, tag="ones")
        ones_bf = pp.tile([128, 1], BF16, tag="ones_bf")
        ones_row = pp.tile([1, 128], F32R, tag="ones_row")
        ident_sb = pp.tile([128, 128], F32, tag="ident")
        bqk_sb = pp.tile([128, 8], F32, tag="bqk")
        vbias_sb = pp.tile([128, HPC], F32, tag="vbias")
        bn_s = pp.tile([128, HPC], F32, tag="bn_s")
        bn_b = pp.tile([128, HPC], F32, tag="bn_b")
        bnc_s = pp.tile([128, HPC], F32, tag="bnc_s")
        bnc_b = pp.tile([128, HPC], F32, tag="bnc_b")
        bnn1_sb = pp.tile([128, 8], F32, tag="bnn1")
        wspe_sb = pp.tile([128, HPC * 128], F32R, tag="wspe")
        wvsum_sb = pp.tile([128, 8 * HPC], F32R, tag="wvsum")
        ident_bf = pp.tile([128, 128], BF16, tag="ident_bf")

        # spe_bc tiles live from phase C1 until the C2 fold
        with tc.tile_pool(name="spb", bufs=4) as spb_pool:
          spe_bcs = []

          # ---- phase C1: conv + spe; vpT := cbr^T ----
          with tc.tile_pool(name="cvin", bufs=2) as cvin_pool, \
               tc.tile_pool(name="cvw", bufs=2) as cvw_pool, \
               tc.tile_pool(name="cbr", bufs=2) as cbr_pool, \
               tc.tile_pool(name="scr", bufs=1) as scr_pool, \
               tc.tile_pool(name="pcol", bufs=2) as pcol_pool, \
               tc.tile_pool(name="cps", bufs=2, space="PSUM") as cps, \
               tc.tile_pool(name="tps", bufs=3, space="PSUM") as tps, \
               tc.tile_pool(name="sps", bufs=1, space="PSUM") as sps:
            xp0 = cvin_pool.tile([128, NPAD], F32R, tag="xp")
            nc.sync.dma_start(xp0[:], xpad_d[0:128, :].bitcast(F32R))
            wcv0 = cvw_pool.tile([128, 9 * 128], F32R, tag="wcv")
            nc.sync.dma_start(wcv0[:], wconv_d[:, 0:9 * 128].bitcast(F32R))
            nc.sync.dma_start(ones_sb[:], ones_d[:].bitcast(F32R))
            nc.vector.tensor_copy(ones_bf[:], ones_sb[:, 0:1].bitcast(F32))
            nc.sync.dma_start(ones_row[:], ones_sb[:, 0:1])
            nc.sync.dma_start(ident_sb[:], ident_d[:])
            nc.sync.dma_start(bqk_sb[:], bqk_d[:])
            nc.sync.dma_start(vbias_sb[:], vbias_d[:])
            nc.sync.dma_start(bn_s[:], bn_s_d[:])
            nc.sync.dma_start(bn_b[:], bn_b_d[:])
            nc.sync.dma_start(bnc_s[:], bnc_s_d[:])
            nc.sync.dma_start(bnc_b[:], bnc_b_d[:])
            nc.sync.dma_start(bnn1_sb[:], bnn1_d[:])
            nc.sync.dma_start(wspe_sb[:], wspe_d[:].bitcast(F32R))
            nc.sync.dma_start(wvsum_sb[:], wvsum_d[:].bitcast(F32R))
            nc.vector.tensor_copy(ident_bf[:], ident_sb[:])

            for h in range(HPC):
                if h == 0:
                    xp, wcv = xp0, wcv0
                else:
                    xp = cvin_pool.tile([128, NPAD], F32R, tag="xp")
                    nc.sync.dma_start(xp[:],
                                      xpad_d[h * 128:(h + 1) * 128, :].bitcast(F32R))
                    wcv = cvw_pool.tile([128, 9 * 128], F32R, tag="wcv")
                    nc.sync.dma_start(
                        wcv[:], wconv_d[:, h * 9 * 128:(h + 1) * 9 * 128].bitcast(F32R))

                # spe branch: gelu(bn(x)) with running row-sum -> pooled -> spe row
                scr = scr_pool.tile([128, N], BF16, tag="scr")
                pcol = pcol_pool.tile([128, 1], F32, tag="pcol")
                interior = xp[:, PP + 1:PP + 1 + PS * PP].rearrange(
                    "p (r c) -> p r c", c=PP)[:, :, 0:PS].bitcast(F32)
                nc.scalar.activation(
                    scr[:].rearrange("p (r c) -> p r c", c=PS), interior,
                    AF.Gelu, bias=bn_b[:, h:h + 1], scale=bn_s[:, h:h + 1],
                    accum_out=pcol[:],
                )
                pcol_r = pcol_pool.tile([128, 1], F32R, tag="pcolr")
                nc.vector.tensor_copy(pcol_r[:], pcol[:])
                ps_spe = sps.tile([1, 128], F32, tag="spe")
                nc.tensor.matmul(ps_spe[:], pcol_r[:],
                                 wspe_sb[:, h * 128:(h + 1) * 128], start=True, stop=True)
                nc.vector.tensor_copy(spe_row[:, h * 128:(h + 1) * 128], ps_spe[:])
                # broadcast spe over partitions: spe_bc[p, c] = spe[c]
                ps_bc = sps.tile([128, 128], F32, tag="bc")
                nc.tensor.matmul(ps_bc[:], ones_row[:],
                                 spe_row[:, h * 128:(h + 1) * 128], start=True, stop=True)
                spe_bc = spb_pool.tile([128, 128], F32, tag="spb")
                nc.vector.tensor_copy(spe_bc[:], ps_bc[:])
                spe_bcs.append(spe_bc)

                # conv branch: 9 shifted matmuls, bn+gelu
                cbr = cbr_pool.tile([128, N], BF16, tag="cbr")
                for rc in range(6):  # 8 output rows per chunk
                    r0 = rc * 8
                    pc = cps.tile([128, 8 * PS], F32, tag="cv")
                    for oi, (dy, dx) in enumerate(
                            [(a, b) for a in (-1, 0, 1) for b in (-1, 0, 1)]):
                        base = (r0 + 1 + dy) * PP + 1 + dx
                        rhs = xp[:, base:base + 8 * PP].rearrange(
                            "p (r c) -> p r c", c=PP)[:, :, 0:PS]
                        nc.tensor.matmul(pc[:].rearrange("p (r c) -> p r c", c=PS),
                                         wcv[:, oi * 128:(oi + 1) * 128], rhs,
                                         start=(oi == 0), stop=(oi == 8))
                    nc.scalar.activation(cbr[:, r0 * PS:(r0 + 8) * PS], pc[:],
                                         AF.Gelu, bias=bnc_b[:, h:h + 1],
                                         scale=bnc_s[:, h:h + 1])
                for jb in range(NJ):
                    pt = tps.tile([128, 128], BF16, tag="tp")
                    nc.tensor.transpose(pt[:], cbr[:, jb * 128:(jb + 1) * 128], ident_bf[:])
                    nc.vector.tensor_copy(
                        vpT_sb[:, (h * NJ + jb) * 128:(h * NJ + jb + 1) * 128], pt[:])

          # ---- phase A: qkv projection (+ folded vsum rows) ----
          with tc.tile_pool(name="wqp", bufs=1) as wqp, \
               tc.tile_pool(name="vr", bufs=1) as vr_pool, \
               tc.tile_pool(name="xch", bufs=4) as xch_pool, \
               tc.tile_pool(name="qps", bufs=4, space="PSUM") as qps, \
               tc.tile_pool(name="vps", bufs=2, space="PSUM") as vps:
            wq_sb = wqp.tile([128, 8 * EB * 128], F32R)
            for dt in range(8):
                nc.sync.dma_start(
                    wq_sb[:, dt * EB * 128:(dt + 1) * EB * 128],
                    wqkv_d[:, dt * EB * 128:(dt + 1) * EB * 128].bitcast(F32R))
            vrow4 = vr_pool.tile([4, N], F32)
            for (n0, nw) in CHUNKS:
                xw = []
                for half in range(2):
                    t = xch_pool.tile([128, 4 * 512], F32R, tag="x")
                    src_ap = x_dn[half * 512:(half + 1) * 512, n0:n0 + nw].rearrange(
                        "(t p) n -> p t n", p=128)
                    nc.sync.dma_start(
                        t[:].rearrange("p (t n) -> p t n", t=4)[:, :, 0:nw],
                        src_ap.bitcast(F32R))
                    xw.append(t)
                xt = [xw[dt // 4][:, (dt % 4) * 512:(dt % 4) * 512 + 512]
                      for dt in range(8)]
                for eb in range(EB):
                    pq = qps.tile([128, 512], F32, tag="q")
                    for dt in range(8):
                        nc.tensor.matmul(
                            pq[:, 0:nw],
                            wq_sb[:, (dt * EB + eb) * 128:(dt * EB + eb + 1) * 128],
                            xt[dt][:, 0:nw], start=(dt == 0), stop=(dt == 7))
                    nc.vector.tensor_scalar_add(
                        qk_sb[:, eb * N + n0:eb * N + n0 + nw],
                        pq[:, 0:nw], bqk_sb[:, eb:eb + 1])
                pv4 = vps.tile([4, 512], F32, tag="v4")
                for dt in range(8):
                    nc.tensor.matmul(pv4[:, 0:nw],
                                     wvsum_sb[:, dt * HPC:(dt + 1) * HPC],
                                     xt[dt][:, 0:nw], start=(dt == 0), stop=(dt == 7))
                nc.vector.tensor_copy(vrow4[:, n0:n0 + nw], pv4[:, 0:nw])

            # ---- phase B: vsum -> per-head columns via DRAM reshape ----
            with tc.tile_pool(name="vdr", bufs=1, space="DRAM") as vdr_pool:
                vdr = vdr_pool.tile([4, N], F32)
                nc.sync.dma_start(vdr[:], vrow4[:])
                for h in range(HPC):
                    nc.sync.dma_start(
                        vcol_all[:, h * NJ:(h + 1) * NJ],
                        vdr[h:h + 1, :].rearrange("o (j p) -> (o p) j", p=128))
                    nc.vector.tensor_scalar_add(
                        vcol_all[:, h * NJ:(h + 1) * NJ],
                        vcol_all[:, h * NJ:(h + 1) * NJ], vbias_sb[:, h:h + 1])

          # ---- phase C2: fold v_spe into V' (in place) ----
          for h in range(HPC):
              for jb in range(NJ):
                  sl = vpT_sb[:, (h * NJ + jb) * 128:(h * NJ + jb + 1) * 128]
                  nc.vector.scalar_tensor_tensor(
                      sl, spe_bcs[h][:], vcol_all[:, h * NJ + jb:h * NJ + jb + 1],
                      sl, MULT, ADD)

        # ---------- phases D+E share outT + wnn ----------
        with tc.tile_pool(name="outp", bufs=1) as outp, \
             tc.tile_pool(name="wnp", bufs=1) as wnp:
            outT_sb = outp.tile([128, HPC * N], F32R)
            wnn_sb = wnp.tile([128, HPC * D], F32R)
            for h in range(HPC):
                nc.sync.dma_start(wnn_sb[:, h * D:(h + 1) * D],
                                  wnn1_d[:, h * D:(h + 1) * D].bitcast(F32R))

            # ---- phase D: attention ----
            with tc.tile_pool(name="pt", bufs=30) as pt_pool, \
                 tc.tile_pool(name="sums", bufs=2) as sum_pool, \
                 tc.tile_pool(name="dps", bufs=4, space="PSUM") as dps, \
                 tc.tile_pool(name="mps", bufs=1, space="PSUM") as mps, \
                 tc.tile_pool(name="ops", bufs=2, space="PSUM") as ops, \
                 tc.tile_pool(name="rps", bufs=1, space="PSUM") as rps:
                for h in range(HPC):
                    qofs, kofs = h * N, (HPC + h) * N
                    for (i0, iw) in CHUNKS:
                        pts = []
                        for jb in range(NJ):
                            pd = dps.tile([128, 512], F32, tag="d")
                            nc.tensor.matmul(pd[:, 0:iw],
                                             qk_sb[:, kofs + jb * 128:kofs + (jb + 1) * 128],
                                             qk_sb[:, qofs + i0:qofs + i0 + iw],
                                             start=True, stop=True)
                            pt = pt_pool.tile([128, 512], BF16, tag="pt")
                            nc.scalar.activation(pt[:, 0:iw], pd[:, 0:iw], AF.Exp,
                                                 scale=SCALE)
                            pts.append(pt)
                        pm = mps.tile([1, 512], F32, tag="m")
                        po = ops.tile([128, 512], F32, tag="o")
                        for jb in range(NJ):
                            nc.tensor.matmul(pm[:, 0:iw], ones_bf[:],
                                             pts[jb][:, 0:iw], start=(jb == 0),
                                             stop=(jb == NJ - 1))
                            nc.tensor.matmul(
                                po[:, 0:iw],
                                vpT_sb[:, (h * NJ + jb) * 128:(h * NJ + jb + 1) * 128],
                                pts[jb][:, 0:iw], start=(jb == 0), stop=(jb == NJ - 1))
                        rsb = sum_pool.tile([1, 512], F32R, tag="r")
                        with nc.allow_low_precision(reason="f32r keeps full fp32 range"):
                            nc.vector.reciprocal(rsb[:, 0:iw], pm[0:1, 0:iw])
                        pr = rps.tile([128, 512], F32, tag="rb")
                        nc.tensor.matmul(pr[:, 0:iw], ones_row[:], rsb[:, 0:iw],
                                         start=True, stop=True)
                        rbs = sum_pool.tile([128, 512], F32, tag="rbs")
                        nc.vector.tensor_copy(rbs[:, 0:iw], pr[:, 0:iw])
                        nc.vector.tensor_tensor(
                            outT_sb[:, h * N + i0:h * N + i0 + iw],
                            po[:, 0:iw], rbs[:, 0:iw], MULT)

            # ---- phase E: nn1 partial + ReduceScatter ----
            with tc.tile_pool(name="fin", bufs=3) as fin_pool, \
                 tc.tile_pool(name="fps", bufs=3, space="PSUM") as fps, \
                 tc.tile_pool(name="dram", bufs=1, space="DRAM") as dram:
                if use_cc:
                    part = dram.tile([D, N], F32)
                    rs0 = dram.tile([CH // 2, N], F32)
                    rs1 = dram.tile([CH // 2, N], F32)
                groups = [[0, 1], [2, 3], [4, 5], [6, 7]]
                for ebo in range(8):
                    fin = fin_pool.tile([128, N], F32, tag="fin")
                    for (n0, nw) in CHUNKS:
                        pf = fps.tile([128, 512], F32, tag="f")
                        for h in range(HPC):
                            nc.tensor.matmul(
                                pf[:, 0:nw],
                                wnn_sb[:, h * D + ebo * 128:h * D + (ebo + 1) * 128],
                                outT_sb[:, h * N + n0:h * N + n0 + nw],
                                start=(h == 0), stop=(h == HPC - 1))
                        nc.scalar.activation(fin[:, n0:n0 + nw], pf[:, 0:nw], AF.Identity,
                                             bias=bnn1_sb[:, ebo:ebo + 1])
                    dst = part if use_cc else out_d
                    nc.sync.dma_start(dst[ebo * 128:(ebo + 1) * 128, :], fin[:])
                    if use_cc and ebo == 3:
                        # overlap first half's pair-reduce with remaining nn1
                        nc.gpsimd.collective_compute(
                            "ReduceScatter", ADD, replica_groups=groups,
                            ins=[part[0:CH, :].opt()], outs=[rs0[:].opt()])
                        nc.sync.dma_start(out_d[0:CH // 2, :], rs0[:])
                if use_cc:
                    nc.gpsimd.collective_compute(
                        "ReduceScatter", ADD, replica_groups=groups,
                        ins=[part[CH:D, :].opt()], outs=[rs1[:].opt()])
                    nc.sync.dma_start(out_d[CH // 2:CH, :], rs1[:])

    nc.compile()
    return nc


def _host_inputs(core, inp):
    b, half = core // 2, core % 2
    h0 = half * HPC
    x = np.asarray(inp["x"][b], dtype=np.float32)            # (D, N)
    Wqkv = np.asarray(inp["Wqkv"], dtype=np.float32)
    bqkv = np.asarray(inp["bqkv"], dtype=np.float32)
    Wspe = np.asarray(inp["Wspe"], dtype=np.float32)[:, :, 0, 0]   # (D, H)
    Wlocal = np.asarray(inp["Wlocal"], dtype=np.float32)     # (D, 8, 3, 3)
    Wnn1 = np.asarray(inp["Wnn1"], dtype=np.float32)
    bnn1 = np.asarray(inp["bnn1"], dtype=np.float32)

    chs = slice(h0 * HD, (h0 + HPC) * HD)                    # this core's 512 channels

    # image layout: reinterpret x^T flat as (D, 48, 48); pad to 50x50
    ximg = np.ascontiguousarray(x.T).reshape(D, N)[chs]      # (512, 2304)
    pad = np.zeros((CH, NPAD), np.float32)
    pad3 = pad[:, :PP * PP].reshape(CH, PP, PP)
    pad3[:, 1:PS + 1, 1:PS + 1] = ximg.reshape(CH, PS, PS)
    xpad = pad

    # qkv weights: e-blocks = [q heads, k heads], lhsT layout; v folded into wvsum
    rows = np.concatenate(
        [np.arange(h0 * HD, (h0 + HPC) * HD) + s * D for s in range(2)])
    wqkvT = Wqkv[rows, :].T                                   # (1024, 1024)
    wq = wqkvT.reshape(8, 128, EB * 128).transpose(1, 0, 2).reshape(128, 8 * EB * 128)
    bqk = bqkv[rows].reshape(8, 128).T.copy()                 # q,k biases (128, 8)
    vrows = np.arange(h0 * HD, (h0 + HPC) * HD) + 2 * D
    wv = Wqkv[vrows, :].reshape(HPC, 128, D).sum(axis=1)      # (HPC, 1024)
    wvsum = wv.T.reshape(8, 128, HPC).transpose(1, 0, 2).reshape(128, 8 * HPC)
    vb = bqkv[vrows].reshape(HPC, 128).sum(axis=1)            # summed v bias per head
    vbias = np.repeat(vb[None, :], 128, axis=0).astype(np.float32)

    # dense per-head conv weights, lhsT[cin, cout] per (head, offset)
    wconv = np.zeros((HPC, 9, 128, 128), np.float32)
    for h in range(HPC):
        for co in range(128):
            g = co // 8
            cg = np.arange(g * 8, g * 8 + 8)
            for oi, (dy, dx) in enumerate(
                    [(a, c) for a in range(3) for c in range(3)]):
                wconv[h, oi, cg, co] = Wlocal[(h0 + h) * HD + co, :, dy, dx]
    wconv = wconv.transpose(2, 0, 1, 3).reshape(128, HPC * 9 * 128)

    # spe block-diag matrix (folds in 1/N pooling mean and attention scale)
    wspe = np.zeros((HPC, 128, 128), np.float32)              # [h, c_in, idx]
    for h in range(HPC):
        for gg in range(16):
            g = (h0 + h) * 16 + gg
            blk = Wspe[g * 8:(g + 1) * 8, :8]                 # [o, i]
            wspe[h, gg * 8:gg * 8 + 8, gg * 8:gg * 8 + 8] = blk.T  # [i, o]
    wspe = (wspe * (SCALE / N)).transpose(1, 0, 2).reshape(128, HPC * 128)

    def fold_bn(g, bta, mu, var):
        s = np.asarray(g, np.float64) / np.sqrt(np.asarray(var, np.float64) + EPS)
        return (s.astype(np.float32),
                (np.asarray(bta, np.float64) - np.asarray(mu, np.float64) * s)
                .astype(np.float32))

    bn_s, bn_b = fold_bn(inp["bn_gamma"], inp["bn_beta"], inp["bn_mean"], inp["bn_var"])
    bnc_s, bnc_b = fold_bn(inp["bnc_gamma"], inp["bnc_beta"], inp["bnc_mean"],
                           inp["bnc_var"])
    shp = lambda a: np.ascontiguousarray(a[chs].reshape(HPC, 128).T)

    wnn1T = Wnn1[:, chs].T                                    # (512, 1024)
    wnn1 = wnn1T.reshape(HPC, 128, D).transpose(1, 0, 2).reshape(128, HPC * D)
    bnn1h = np.ascontiguousarray((0.5 * bnn1).reshape(8, 128).T)

    ones = np.ones((128, 2), np.float32)
    ident = np.eye(128, dtype=np.float32)
    return {
        "x_dn": np.ascontiguousarray(x), "xpad": xpad,
        "wqkv": np.ascontiguousarray(wq), "bqk": np.ascontiguousarray(bqk),
        "wvsum": np.ascontiguousarray(wvsum),
        "vbias": vbias, "wconv": np.ascontiguousarray(wconv),
        "wspe": np.ascontiguousarray(wspe),
        "bn_s": shp(bn_s), "bn_b": shp(bn_b), "bnc_s": shp(bnc_s), "bnc_b": shp(bnc_b),
        "wnn1": np.ascontiguousarray(wnn1), "bnn1h": bnn1h,
        "onesc": ones, "ident": ident,
    }


_NC = None


def kernel(**inputs):
    global _NC
    if _NC is None:
        _NC = _build()
    in_maps = [_host_inputs(c, inputs) for c in range(8)]
    res = run_bass_kernel_spmd(_NC, in_maps, core_ids=list(range(8)))
    out = np.empty((B, N, D), np.float32)
    for b in range(B):
        if USE_COLLECTIVE:
            ev, od = res.results[2 * b]["out"], res.results[2 * b + 1]["out"]
            t = np.empty((D, N), np.float32)
            t[0:256] = ev[0:256]
            t[256:512] = od[0:256]
            t[512:768] = ev[256:512]
            t[768:1024] = od[256:512]
        else:
            t = res.results[2 * b]["out"] + res.results[2 * b + 1]["out"]
        out[b] = t.T
    return out


def run_timed(**inputs):
    """Re-run with NTFF tracing to get HW exec time (best effort)."""
    global _NC
    if _NC is None:
        _NC = _build()
    in_maps = [_host_inputs(c, inputs) for c in range(8)]
    try:
        return run_bass_kernel_spmd(_NC, in_maps, core_ids=list(range(8)), trace=True)
    except Exception as e:  # tracing unsupported under some axon terminals
        print(f"trace run failed: {e}")
        return None



# revision 9
# speedup vs baseline: 1.2916x; 1.2916x over previous
"""Trainium2 Bass kernel for nn_Attention_87857851006980.

Sharding: 8 cores = 4 batches x 2 head-halves. Core c handles batch c//2,
heads [0..4) (even c) or [4..8) (odd c). Each core computes qkv for its
heads (full-d contraction), the conv/spe branches for its heads' channels,
attention for its heads, and a partial nn1 over its 512 channels; a
pair-wise ReduceScatter then sums the nn1 partials, leaving each core with
half of the output channels for its batch. Host gathers/transposes.

v2: fp8 DoubleRow matmuls for qkv / dots / softmax-sum / attn@V / conv
(2x contraction per pass), inline per-j-block v-sum (no DRAM roundtrip),
bf16 nn1, batched exp on the scalar engine, gpsimd (Pool) engine offload
for copies and partition broadcasts. Host only reshapes, quantizes dtypes,
and folds BN/bias constants.
"""
import sys
sys.path.insert(0, "/opt/trn_rl_repo")
import numpy as np
import ml_dtypes

import concourse.bacc as bacc
import concourse.bass as bass
import concourse.bass_isa as bass_isa
import concourse.tile as tile
import concourse.mybir as mybir
from concourse.bass_utils import run_bass_kernel_spmd

F32 = mybir.dt.float32
F32R = mybir.dt.float32r
BF16 = mybir.dt.bfloat16
FP8 = mybir.dt.float8e4
AF = mybir.ActivationFunctionType
ADD = mybir.AluOpType.add
MULT = mybir.AluOpType.mult
DRM = mybir.MatmulPerfMode.DoubleRow
RADD = bass_isa.ReduceOp.add
E4 = ml_dtypes.float8_e4m3
BF = ml_dtypes.bfloat16

B, D, N, H, HD = 4, 1024, 2304, 8, 128
PS = 48
HPC = 4          # heads per core
CH = HPC * HD    # 512 channels per core
NJ = 18          # key blocks of 128
EPS = 1e-5
SCALE = D ** -0.5
QS = 8.0         # q/k weight prescale (fp8 range centering)
CVS = 4.0        # conv weight prescale
EXPSCALE = SCALE / (QS * QS)
PLANE = 2400     # 48 zeros + 2304 image + 48 zeros
CHUNKS = [(0, 512), (512, 512), (1024, 512), (1536, 512), (2048, 256)]

# conv offsets (dy,dx) variant-major; 5 DoubleRow pair-groups (last
# group's second slot has zero weights and re-reads offset 8).
OFFS = [(dy, dx) for dx in (-1, 0, 1) for dy in (-1, 0, 1)]
CVGROUPS = [(0, 1), (2, 3), (4, 5), (6, 7), (8, 8)]

USE_COLLECTIVE = True


def _ap3(t, off, s1, n1, s2, n2):
    """Raw 3D AP [partitions, (s1,n1), (s2,n2)] at element offset off into
    tile t (supports overlapping / arbitrary strides)."""
    base = t[:]
    p = base.ap[0]
    return bass.AP(base.tensor, base.offset + off, [list(p), [s1, n1], [s2, n2]])


def _build(single=False, gelu=True):
    use_cc = USE_COLLECTIVE and not single
    nc = bacc.Bacc("TRN2", target_bir_lowering=False, debug=False,
                   num_devices=1 if single else 8)
    AFG = AF.Gelu if gelu else AF.Identity

    xq_d = nc.dram_tensor("xq", [128, 8 * N], FP8, kind="ExternalInput").ap()
    xvar_d = nc.dram_tensor("xvar", [128, HPC * 3 * PLANE + 128], FP8, kind="ExternalInput").ap()
    wq_d = nc.dram_tensor("wq", [128, 64 * 128], FP8, kind="ExternalInput").ap()
    wv_d = nc.dram_tensor("wv", [128, 8 * HPC], FP8, kind="ExternalInput").ap()
    bqk_d = nc.dram_tensor("bqk64", [64, 16], F32, kind="ExternalInput").ap()
    vb72_d = nc.dram_tensor("vb72", [128, NJ * HPC], F32, kind="ExternalInput").ap()
    wcv_d = nc.dram_tensor("wcv", [128, HPC * 5 * 256], FP8, kind="ExternalInput").ap()
    wspe_d = nc.dram_tensor("wspe", [128, HPC * 128], F32, kind="ExternalInput").ap()
    bn_s_d = nc.dram_tensor("bn_s", [128, HPC], F32, kind="ExternalInput").ap()
    bn_b_d = nc.dram_tensor("bn_b", [128, HPC], F32, kind="ExternalInput").ap()
    bnc_s_d = nc.dram_tensor("bnc_s", [128, HPC], F32, kind="ExternalInput").ap()
    bnc_b_d = nc.dram_tensor("bnc_b", [128, HPC], F32, kind="ExternalInput").ap()
    wnn_d = nc.dram_tensor("wnn", [128, HPC * D], BF16, kind="ExternalInput").ap()
    bnn1_d = nc.dram_tensor("bnn1h", [128, 8], F32, kind="ExternalInput").ap()
    ones_d = nc.dram_tensor("ones8", [128, 32], FP8, kind="ExternalInput").ap()
    if use_cc:
        out_d = nc.dram_tensor("out", [CH, N], F32, kind="ExternalOutput").ap()
    else:
        out_d = nc.dram_tensor("out", [D, N], F32, kind="ExternalOutput").ap()

    with tile.TileContext(nc) as tc:
      with tc.tile_pool(name="persist", bufs=1) as pp:
        xq = pp.tile([128, 8 * N], FP8, tag="xq")
        xvar = pp.tile([128, HPC * 3 * PLANE + 128], FP8, tag="xvar")
        wq = pp.tile([128, 64 * 128], FP8, tag="wq")
        wv = pp.tile([128, 8 * HPC], FP8, tag="wv")
        bqk = pp.tile([64, 16], F32, tag="bqk")
        vb72 = pp.tile([128, NJ * HPC], F32, tag="vb72")
        wcv = pp.tile([128, HPC * 5 * 256], FP8, tag="wcv")
        wspe = pp.tile([128, HPC * 128], F32, tag="wspe")
        bn_s = pp.tile([128, HPC], F32, tag="bn_s")
        bn_b = pp.tile([128, HPC], F32, tag="bn_b")
        bnc_s = pp.tile([128, HPC], F32, tag="bnc_s")
        bnc_b = pp.tile([128, HPC], F32, tag="bnc_b")
        wnn = pp.tile([128, HPC * D], BF16, tag="wnn")
        bnn1 = pp.tile([128, 8], F32, tag="bnn1")
        ones8 = pp.tile([128, 32], FP8, tag="ones8")
        qk64 = pp.tile([64, 16 * N], FP8, tag="qk64")
        vpT = pp.tile([128, HPC * NJ * 128], FP8, tag="vpT")
        vcol = pp.tile([128, NJ * HPC], F32, tag="vcol")
        outT = pp.tile([128, HPC * N], BF16, tag="outT")
        spe_bc = pp.tile([128, HPC * 128], F32, tag="spe_bc")

        nc.sync.dma_start(xvar[:], xvar_d[:])
        nc.sync.dma_start(xq[:], xq_d[:])
        nc.sync.dma_start(wq[:], wq_d[:])
        nc.sync.dma_start(wv[:], wv_d[:])
        nc.sync.dma_start(bqk[:], bqk_d[:])
        nc.sync.dma_start(vb72[:], vb72_d[:])
        nc.sync.dma_start(bn_s[:], bn_s_d[:])
        nc.sync.dma_start(bn_b[:], bn_b_d[:])
        nc.sync.dma_start(wspe[:], wspe_d[:])
        nc.sync.dma_start(wcv[:], wcv_d[:])
        nc.sync.dma_start(bnc_s[:], bnc_s_d[:])
        nc.sync.dma_start(bnc_b[:], bnc_b_d[:])
        nc.sync.dma_start(ones8[:], ones_d[:])
        nc.sync.dma_start(wnn[:], wnn_d[:])
        nc.sync.dma_start(bnn1[:], bnn1_d[:])

        xqv = xq[:].rearrange("p (t n) -> p t n", t=8)
        wvv = wv[:].rearrange("p (t h) -> p t h", t=8)
        qkv_ = qk64[:].rearrange("p (b n) -> p b n", b=16)

        with tc.tile_pool(name="ptp", bufs=30) as pt_pool, \
             tc.tile_pool(name="sums", bufs=2) as sum_pool, \
             tc.tile_pool(name="spep", bufs=2) as spe_pool, \
             tc.tile_pool(name="cbrp", bufs=2) as cbr_pool, \
             tc.tile_pool(name="finp", bufs=2) as fin_pool, \
             tc.tile_pool(name="dram", bufs=1, space="DRAM") as dram_pool:
            if use_cc:
                part = dram_pool.tile([D, N], F32, name="part")
                rs0 = dram_pool.tile([CH // 2, N], F32, name="rs0")
                rs1 = dram_pool.tile([CH // 2, N], F32, name="rs1")
            else:
                part = None

            pt_tiles = {}

            def qkv_block(qps, blk, c):
                n0, nw = CHUNKS[c]
                pq = qps.tile([64, 512], F32, tag="q", name="pq")
                for t in range(4):
                    nc.tensor.matmul(
                        pq[:, 0:nw],
                        wq[:, (t * 16 + blk) * 128:(t * 16 + blk + 1) * 128]
                        .rearrange("p (i m) -> p i m", i=2),
                        xqv[:, 2 * t:2 * t + 2, n0:n0 + nw],
                        start=(t == 0), stop=(t == 3), perf_mode=DRM)
                nc.vector.tensor_scalar_add(qkv_[:, blk, n0:n0 + nw],
                                            pq[:, 0:nw], bqk[:, blk:blk + 1])

            def pd_batch(dps, c, h, p):
                n0, nw = CHUNKS[c]
                dp = dps.tile([128, 1024], F32, tag="d", name="dp")
                for i in range(2):
                    jb = 2 * p + i
                    nc.tensor.matmul(
                        dp[:, i * nw:(i + 1) * nw],
                        qkv_[:, 2 * (4 + h):2 * (4 + h) + 2,
                             jb * 128:(jb + 1) * 128],
                        qkv_[:, 2 * h:2 * h + 2, n0:n0 + nw],
                        start=True, stop=True, perf_mode=DRM)
                pt = pt_pool.tile([128, 1024], FP8, tag="pt", name="pt")
                nc.scalar.activation(pt[:, 0:2 * nw], dp[:, 0:2 * nw], AF.Exp,
                                     scale=EXPSCALE)
                pt_tiles[(c, h)].append(pt)

            def pmpo_batch(pm, po, c, h, p):
                n0, nw = CHUNKS[c]
                pt = pt_tiles[(c, h)][p]
                ptv = pt[:, 0:2 * nw].rearrange("p (i n) -> p i n", i=2)
                nc.tensor.matmul(pm[0:1, 0:nw],
                                 _ap3(ones8, 0, 16, 2, 1, 1),
                                 ptv, start=(p == 0), stop=(p == 8),
                                 perf_mode=DRM)
                nc.tensor.matmul(
                    po[:, 0:nw],
                    vpT[:, (h * NJ + 2 * p) * 128:(h * NJ + 2 * p + 2) * 128]
                    .rearrange("p (i m) -> p i m", i=2),
                    ptv, start=(p == 0), stop=(p == 8), perf_mode=DRM)

            def finish_head(pm, po, c, h):
                n0, nw = CHUNKS[c]
                rsb = sum_pool.tile([1, 512], F32, tag="rsb", name="rsb")
                with nc.allow_low_precision(reason="softmax denominators"):
                    nc.vector.reciprocal(rsb[0:1, 0:nw], pm[0:1, 0:nw])
                rbs = sum_pool.tile([128, 512], F32, tag="rbs", name="rbs")
                nc.gpsimd.partition_broadcast(rbs[:, 0:nw], rsb[0:1, 0:nw])
                nc.vector.tensor_tensor(outT[:, h * N + n0:h * N + n0 + nw],
                                        po[:, 0:nw], rbs[:, 0:nw], MULT)
                pt_tiles.pop((c, h))

            def spe_head(h):
                pcol = spe_pool.tile([128, 1], F32, tag="pcol", name="pcol")
                ctr = h * 3 * PLANE + PLANE + 48
                nc.scalar.activation(outT[:, 0:N], xvar[:, ctr:ctr + N], AFG,
                                     bias=bn_b[:, h:h + 1],
                                     scale=bn_s[:, h:h + 1], accum_out=pcol[:])
                tmp = spe_pool.tile([128, 128], F32, tag="spetmp", name="tmp")
                nc.gpsimd.tensor_scalar_mul(tmp[:],
                                            wspe[:, h * 128:(h + 1) * 128],
                                            pcol[:])
                nc.gpsimd.partition_all_reduce(
                    spe_bc[:, h * 128:(h + 1) * 128], tmp[:], 128, RADD)

            def conv_batch(cps, h, bt):
                """3 j-blocks of the conv for head h, output directly in
                [n, c] (vpT) orientation; BN+gelu folded; v' fold on DVE."""
                hb = h * 3 * PLANE
                bias_off = HPC * 3 * PLANE
                pcT = cps.tile([128, 384], F32, tag="cv", name="pcT")
                for sub in range(3):
                    jb = bt * 3 + sub
                    for g, (o1, o2) in enumerate(CVGROUPS):
                        dy1, dx1 = OFFS[o1]
                        off1 = hb + (dx1 + 1) * PLANE + 48 + dy1 * 48 + jb * 128
                        if g < 4:
                            dy2, dx2 = OFFS[o2]
                            off2 = (hb + (dx2 + 1) * PLANE + 48 + dy2 * 48
                                    + jb * 128)
                        else:
                            off2 = bias_off
                        nc.tensor.matmul(
                            pcT[:, sub * 128:(sub + 1) * 128],
                            _ap3(xvar, off1, off2 - off1, 2, 1, 128),
                            wcv[:, (h * 5 + g) * 256:(h * 5 + g + 1) * 256]
                            .rearrange("p (i m) -> p i m", i=2),
                            start=(g == 0), stop=(g == 4), perf_mode=DRM)
                cbrT = cbr_pool.tile([128, 384], FP8, tag="cbrT", name="cbrT")
                nc.scalar.activation(cbrT[:], pcT[:], AFG, scale=1.0 / CVS)
                for sub in range(3):
                    jb = bt * 3 + sub
                    nc.vector.scalar_tensor_tensor(
                        vpT[:, (h * NJ + jb) * 128:(h * NJ + jb + 1) * 128],
                        spe_bc[:, h * 128:(h + 1) * 128],
                        vcol[:, jb * HPC + h:jb * HPC + h + 1],
                        cbrT[:, sub * 128:(sub + 1) * 128], MULT, ADD)

            def nn1_chunk(fps, c):
                n0, nw = CHUNKS[c]
                for ebo in range(8):
                    pf = fps.tile([128, 512], F32, tag="f", name="pf")
                    for h in range(HPC):
                        nc.tensor.matmul(
                            pf[:, 0:nw],
                            wnn[:, h * D + ebo * 128:h * D + (ebo + 1) * 128],
                            outT[:, h * N + n0:h * N + n0 + nw],
                            start=(h == 0), stop=(h == HPC - 1))
                    fin = fin_pool.tile([128, 512], F32, tag="fin", name="fin")
                    nc.vector.tensor_scalar_add(fin[:, 0:nw], pf[:, 0:nw],
                                                bnn1[:, ebo:ebo + 1])
                    dst = part if use_cc else out_d
                    nc.sync.dma_start(dst[ebo * 128:(ebo + 1) * 128,
                                          n0:n0 + nw], fin[:, 0:nw])

            # ACT stream starts with the 4 pooling gelus (spe path)
            for h in range(HPC):
                spe_head(h)

            with tc.tile_pool(name="dps", bufs=2, space="PSUM") as dps:
                # ---- WA: qkv + conv + chunk-0 dots, finely interleaved ----
                with tc.tile_pool(name="qps", bufs=2, space="PSUM") as qps, \
                     tc.tile_pool(name="vps", bufs=1, space="PSUM") as vps, \
                     tc.tile_pool(name="cps", bufs=1, space="PSUM") as cps:
                    # filler work-queue: popped between pd batches
                    work = [("vcol",)]
                    for h in range(HPC):
                        for bt in range(6):
                            work.append(("conv", h, bt))
                    for c in range(1, 5):
                        for blk in range(0, 8):
                            work.append(("qkv", blk, c))
                    wi = 0

                    def do_work(n):
                        nonlocal wi
                        for _ in range(n):
                            if wi >= len(work):
                                return
                            w = work[wi]
                            wi += 1
                            if w[0] == "qkv":
                                qkv_block(qps, w[1], w[2])
                            elif w[0] == "conv":
                                conv_batch(cps, w[1], w[2])
                            else:
                                pv = vps.tile([128, NJ * HPC], F32, name="pv")
                                for jb in range(NJ):
                                    for t in range(4):
                                        nc.tensor.matmul(
                                            pv[:, jb * HPC:(jb + 1) * HPC],
                                            xqv[:, 2 * t:2 * t + 2,
                                                jb * 128:(jb + 1) * 128],
                                            wvv[:, 2 * t:2 * t + 2, :],
                                            start=(t == 0), stop=(t == 3),
                                            perf_mode=DRM)
                                nc.vector.tensor_tensor(vcol[:], pv[:],
                                                        vb72[:], ADD)

                    # per head: its k blocks (all chunks) + q blocks (chunk
                    # 0) first, then its chunk-0 dots with filler between
                    for h in range(HPC):
                        for c in range(5):
                            qkv_block(qps, 8 + 2 * h, c)
                            qkv_block(qps, 9 + 2 * h, c)
                        qkv_block(qps, 2 * h, 0)
                        qkv_block(qps, 2 * h + 1, 0)
                        pt_tiles[(0, h)] = []
                        for p in range(9):
                            pd_batch(dps, 0, h, p)
                            do_work(2)
                    do_work(len(work))

                # ---- WC: steady-state attention + nn1 ----
                with tc.tile_pool(name="pms", bufs=1, space="PSUM") as pms, \
                     tc.tile_pool(name="pos", bufs=2, space="PSUM") as pos, \
                     tc.tile_pool(name="fps", bufs=1, space="PSUM") as fps:
                    for h in range(HPC):
                        pm = pms.tile([1, 512], F32, tag="m", name="pm")
                        po = pos.tile([128, 512], F32, tag="o", name="po")
                        for p in range(9):
                            pmpo_batch(pm, po, 0, h, p)
                        finish_head(pm, po, 0, h)
                    for c in range(1, 5):
                        for h in range(HPC):
                            pt_tiles[(c, h)] = []
                            for p in range(0, 3):
                                pd_batch(dps, c, h, p)
                            pm = pms.tile([1, 512], F32, tag="m", name="pm")
                            po = pos.tile([128, 512], F32, tag="o", name="po")
                            for p in range(0, 3):
                                pmpo_batch(pm, po, c, h, p)
                            for p in range(3, 9):
                                pd_batch(dps, c, h, p)
                                pmpo_batch(pm, po, c, h, p)
                            finish_head(pm, po, c, h)
                            if h == 3:
                                nn1_chunk(fps, c - 1)
                    nn1_chunk(fps, 4)

                    if use_cc:
                        groups = [[0, 1], [2, 3], [4, 5], [6, 7]]
                        nc.gpsimd.collective_compute(
                            "ReduceScatter", ADD, replica_groups=groups,
                            ins=[part[0:CH, :].opt()], outs=[rs0[:].opt()])
                        nc.sync.dma_start(out_d[0:CH // 2, :], rs0[:])
                        nc.gpsimd.collective_compute(
                            "ReduceScatter", ADD, replica_groups=groups,
                            ins=[part[CH:D, :].opt()], outs=[rs1[:].opt()])
                        nc.sync.dma_start(out_d[CH // 2:CH, :], rs1[:])

    nc.compile()
    return nc


def _host_inputs(core, inp):
    b, half = core // 2, core % 2
    h0 = half * HPC
    x = np.asarray(inp["x"][b], dtype=np.float32)            # (D, N)
    Wqkv = np.asarray(inp["Wqkv"], dtype=np.float32)
    bqkv = np.asarray(inp["bqkv"], dtype=np.float32)
    Wspe = np.asarray(inp["Wspe"], dtype=np.float32)[:, :, 0, 0]   # (D, H)
    Wlocal = np.asarray(inp["Wlocal"], dtype=np.float32)     # (D, 8, 3, 3)
    Wnn1 = np.asarray(inp["Wnn1"], dtype=np.float32)
    bnn1 = np.asarray(inp["bnn1"], dtype=np.float32)

    chs = slice(h0 * HD, (h0 + HPC) * HD)

    def fold_bn(g, bta, mu, var):
        s = np.asarray(g, np.float64) / np.sqrt(np.asarray(var, np.float64) + EPS)
        return (s.astype(np.float32),
                (np.asarray(bta, np.float64) - np.asarray(mu, np.float64) * s)
                .astype(np.float32))


    # qkv moving operand: [p, dt, n]
    xq = np.ascontiguousarray(x.reshape(8, 128, N).transpose(1, 0, 2)
                              .reshape(128, 8 * N)).astype(E4)

    # conv image: raw reinterpret of x^T as (D, 48, 48); 3 dx-shifted
    # variants with 48-zero top/bottom pads, flattened per head-block
    ximg = np.ascontiguousarray(x.T).reshape(D, N)[chs]      # (512, 2304)
    xvar = np.zeros((128, HPC * 3 * PLANE + 128), np.float32)
    xv4 = xvar[:, :HPC * 3 * PLANE].reshape(128, HPC, 3, PLANE)
    img4 = ximg.reshape(HPC, 128, PS, PS)
    for hb in range(HPC):
        for vi, dx in enumerate((-1, 0, 1)):
            sh = np.zeros((128, PS, PS), np.float32)
            if dx == -1:
                sh[:, :, 1:] = img4[hb, :, :, :-1]
            elif dx == 1:
                sh[:, :, :-1] = img4[hb, :, :, 1:]
            else:
                sh = img4[hb]
            xv4[:, hb, vi, 48:48 + N] = sh.reshape(128, N)
    xvar[:, HPC * 3 * PLANE:] = 1.0 / 32.0          # conv bias plane
    xvar = np.ascontiguousarray(xvar).astype(E4)

    # q/k weights: 16 eb64 blocks (q0..q3,k0..k3 x dhalf), x8 prescale
    rows = np.concatenate(
        [np.arange(h0 * HD, (h0 + HPC) * HD) + s * D for s in range(2)])
    W8 = Wqkv[rows, :] * QS                                   # (1024, 1024)
    # wq[p, (t*16+blk)*128 + i*64 + m] = W8[blk*64+m, (2t+i)*128+p]
    wq = W8.reshape(16, 64, 4, 2, 128).transpose(4, 2, 0, 3, 1)
    wq = np.ascontiguousarray(wq.reshape(128, 64 * 128)).astype(E4)
    bqk64 = np.ascontiguousarray((QS * bqkv[rows]).reshape(16, 64).T
                                 .astype(np.float32))

    # v-sum weights [p, t, h]; bias pre-tiled [p, (jb, h)]
    vrows = np.arange(h0 * HD, (h0 + HPC) * HD) + 2 * D
    wvs = Wqkv[vrows, :].reshape(HPC, 128, D).sum(axis=1)     # (HPC, 1024)
    wvh = wvs.T.reshape(8, 128, HPC).transpose(1, 0, 2)
    wv = np.ascontiguousarray(wvh.reshape(128, 8 * HPC)).astype(E4)
    vb = bqkv[vrows].reshape(HPC, 128).sum(axis=1)
    vb72 = np.ascontiguousarray(
        np.tile(vb[None, None, :], (128, NJ, 1)).reshape(128, NJ * HPC)
        .astype(np.float32))

    # dense per-head conv weights, DoubleRow pair groups, x4 prescale
    wconv = np.zeros((HPC, 9, 128, 128), np.float32)
    for h in range(HPC):
        for co in range(128):
            g = co // 8
            cg = np.arange(g * 8, g * 8 + 8)
            for oi, (dy, dx) in enumerate(OFFS):
                wconv[h, oi, cg, co] = Wlocal[(h0 + h) * HD + co, :, dy + 1, dx + 1]
    # bn scale folded into weights; bias via the 1/32 plane (x32 here)
    bnc_s_full, bnc_b_full = fold_bn(inp["bnc_gamma"], inp["bnc_beta"],
                                     inp["bnc_mean"], inp["bnc_var"])
    sc = bnc_s_full[chs].reshape(HPC, 128)            # per (h, c)
    bc = bnc_b_full[chs].reshape(HPC, 128)
    wcv = np.zeros((128, HPC, 5, 2, 128), np.float32)
    for h in range(HPC):
        for g, (o1, o2) in enumerate(CVGROUPS):
            wcv[:, h, g, 0, :] = CVS * wconv[h, o1] * sc[h][None, :]
            if g < 4:
                wcv[:, h, g, 1, :] = CVS * wconv[h, o2] * sc[h][None, :]
            else:
                # encode 32*CVS*bc over 4 rows with residual refinement so
                # the (systematic) bias survives fp8 quantization
                S = 32.0 * CVS * bc[h]
                acc = np.zeros_like(S)
                for r in range(4):
                    step = np.asarray((S - acc) if r else S / 2, np.float32)
                    q = step.astype(E4).astype(np.float32)
                    wcv[r, h, g, 1, :] = q
                    acc += q
    wcv = np.ascontiguousarray(wcv.reshape(128, HPC * 5 * 256)).astype(E4)

    # spe block-diag matrix (folds 1/N pooling mean and attention scale)
    wspe = np.zeros((HPC, 128, 128), np.float32)
    for h in range(HPC):
        for gg in range(16):
            g = (h0 + h) * 16 + gg
            blk = Wspe[g * 8:(g + 1) * 8, :8]
            wspe[h, gg * 8:gg * 8 + 8, gg * 8:gg * 8 + 8] = blk.T
    wspe = np.ascontiguousarray(
        (wspe * (SCALE / N)).transpose(1, 0, 2).reshape(128, HPC * 128))

    bn_s, bn_b = fold_bn(inp["bn_gamma"], inp["bn_beta"], inp["bn_mean"],
                         inp["bn_var"])
    bnc_s, bnc_b = fold_bn(inp["bnc_gamma"], inp["bnc_beta"], inp["bnc_mean"],
                           inp["bnc_var"])
    shp = lambda a: np.ascontiguousarray(a[chs].reshape(HPC, 128).T)

    wnn1T = Wnn1[:, chs].T                                    # (512, 1024)
    wnn = np.ascontiguousarray(
        wnn1T.reshape(HPC, 128, D).transpose(1, 0, 2).reshape(128, HPC * D)
    ).astype(BF)
    bnn1h = np.ascontiguousarray((0.5 * bnn1).reshape(8, 128).T)

    return {
        "xq": xq, "xvar": xvar, "wq": wq, "wv": wv,
        "bqk64": bqk64, "vb72": vb72,
        "wcv": wcv, "wspe": wspe,
        "bn_s": shp(bn_s), "bn_b": shp(bn_b),
        "bnc_s": shp(bnc_s / CVS), "bnc_b": shp(bnc_b),
        "wnn": wnn, "bnn1h": bnn1h,
        "ones8": _ones_pair(),
    }


def _ones_pair():
    o = np.zeros((128, 32), np.float32)
    o[:, 0] = 1.0
    o[:, 16] = 1.0
    return o.astype(E4)


_NC = None


def kernel(**inputs):
    global _NC
    if _NC is None:
        _NC = _build()
    in_maps = [_host_inputs(c, inputs) for c in range(8)]
    res = run_bass_kernel_spmd(_NC, in_maps, core_ids=list(range(8)))
    out = np.empty((B, N, D), np.float32)
    for b in range(B):
        if USE_COLLECTIVE:
            ev, od = res.results[2 * b]["out"], res.results[2 * b + 1]["out"]
            t = np.empty((D, N), np.float32)
            t[0:256] = ev[0:256]
            t[256:512] = od[0:256]
            t[512:768] = ev[256:512]
            t[768:1024] = od[256:512]
        else:
            t = res.results[2 * b]["out"] + res.results[2 * b + 1]["out"]
        out[b] = t.T
    return out


def run_timed(**inputs):
    """Re-run with NTFF tracing to get HW exec time (best effort)."""
    global _NC
    if _NC is None:
        _NC = _build()
    in_maps = [_host_inputs(c, inputs) for c in range(8)]
    try:
        return run_bass_kernel_spmd(_NC, in_maps, core_ids=list(range(8)), trace=True)
    except Exception as e:
        print(f"trace run failed: {e}")
        return None


# revision 32
# speedup vs baseline: 1.4546x; 1.1262x over previous
"""Trainium2 Bass kernel for nn_Attention_87857851006980.

Sharding: 8 cores = 4 batches x 2 head-halves. Core c handles batch c//2,
heads [0..4) (even c) or [4..8) (odd c). Each core computes qkv for its
heads (full-d contraction), the conv/spe branches for its heads' channels,
attention for its heads, and a partial nn1 over its 512 channels; a
pair-wise ReduceScatter then sums the nn1 partials, leaving each core with
half of the output channels for its batch. Host gathers/transposes.

v2: fp8 DoubleRow matmuls for qkv / dots / softmax-sum / attn@V / conv
(2x contraction per pass), inline per-j-block v-sum (no DRAM roundtrip),
bf16 nn1, batched exp on the scalar engine, gpsimd (Pool) engine offload
for copies and partition broadcasts. Host only reshapes, quantizes dtypes,
and folds BN/bias constants.
"""
import sys
sys.path.insert(0, "/opt/trn_rl_repo")
import numpy as np
import ml_dtypes

import concourse.bacc as bacc
import concourse.bass as bass
import concourse.bass_isa as bass_isa
import concourse.tile as tile
import concourse.mybir as mybir
from concourse.bass_utils import run_bass_kernel_spmd

F32 = mybir.dt.float32
F32R = mybir.dt.float32r
BF16 = mybir.dt.bfloat16
FP8 = mybir.dt.float8e4
AF = mybir.ActivationFunctionType
ADD = mybir.AluOpType.add
MULT = mybir.AluOpType.mult
DRM = mybir.MatmulPerfMode.DoubleRow
RADD = bass_isa.ReduceOp.add
E4 = ml_dtypes.float8_e4m3
BF = ml_dtypes.bfloat16

B, D, N, H, HD = 4, 1024, 2304, 8, 128
PS = 48
HPC = 4          # heads per core
CH = HPC * HD    # 512 channels per core
NJ = 18          # key blocks of 128
EPS = 1e-5
SCALE = D ** -0.5
QS = 8.0         # q/k weight prescale (fp8 range centering)
CVS = 4.0        # conv weight prescale
EXPSCALE = SCALE / (QS * QS)
PLANE = 2400     # 48 zeros + 2304 image + 48 zeros
CHUNKS = [(0, 512), (512, 512), (1024, 512), (1536, 512), (2048, 256)]

# conv offsets (dy,dx) variant-major; 5 DoubleRow pair-groups (last
# group's second slot has zero weights and re-reads offset 8).
OFFS = [(dy, dx) for dx in (-1, 0, 1) for dy in (-1, 0, 1)]
CVGROUPS = [(0, 1), (2, 3), (4, 5), (6, 7), (8, 8)]

USE_COLLECTIVE = True


def _ap3(t, off, s1, n1, s2, n2):
    """Raw 3D AP [partitions, (s1,n1), (s2,n2)] at element offset off into
    tile t (supports overlapping / arbitrary strides)."""
    base = t[:]
    p = base.ap[0]
    return bass.AP(base.tensor, base.offset + off, [list(p), [s1, n1], [s2, n2]])


def _build(single=False, gelu=True):
    use_cc = USE_COLLECTIVE and not single
    nc = bacc.Bacc("TRN2", target_bir_lowering=False, debug=False,
                   num_devices=1 if single else 8)
    AFG = AF.Gelu if gelu else AF.Identity

    xq_d = nc.dram_tensor("xq", [128, 8 * N], FP8, kind="ExternalInput").ap()
    xvar_d = nc.dram_tensor("xvar", [128, HPC * 3 * PLANE + 128], FP8, kind="ExternalInput").ap()
    wq_d = nc.dram_tensor("wq", [128, 64 * 128], FP8, kind="ExternalInput").ap()
    wv_d = nc.dram_tensor("wv", [128, 8 * HPC], FP8, kind="ExternalInput").ap()
    bqk_d = nc.dram_tensor("bqk64", [64, 16], F32, kind="ExternalInput").ap()
    vb72_d = nc.dram_tensor("vb72", [128, NJ * HPC], F32, kind="ExternalInput").ap()
    wcv_d = nc.dram_tensor("wcv", [128, HPC * 5 * 256], FP8, kind="ExternalInput").ap()
    wspe_d = nc.dram_tensor("wspe", [128, HPC * 128], F32, kind="ExternalInput").ap()
    bn_s_d = nc.dram_tensor("bn_s", [128, HPC], F32, kind="ExternalInput").ap()
    bn_b_d = nc.dram_tensor("bn_b", [128, HPC], F32, kind="ExternalInput").ap()
    bnc_s_d = nc.dram_tensor("bnc_s", [128, HPC], F32, kind="ExternalInput").ap()
    bnc_b_d = nc.dram_tensor("bnc_b", [128, HPC], F32, kind="ExternalInput").ap()
    wnn_d = nc.dram_tensor("wnn", [128, HPC * D], BF16, kind="ExternalInput").ap()
    bnn1_d = nc.dram_tensor("bnn1h", [128, 8], F32, kind="ExternalInput").ap()
    ones_d = nc.dram_tensor("ones8", [128, 32], FP8, kind="ExternalInput").ap()
    if use_cc:
        out_d = nc.dram_tensor("out", [CH, N], F32, kind="ExternalOutput").ap()
    else:
        out_d = nc.dram_tensor("out", [D, N], F32, kind="ExternalOutput").ap()

    with tile.TileContext(nc) as tc:
      with tc.tile_pool(name="persist", bufs=1) as pp:
        xq = pp.tile([128, 8 * N], FP8, tag="xq")
        xvar = pp.tile([128, HPC * 3 * PLANE + 128], FP8, tag="xvar")
        wq = pp.tile([128, 64 * 128], FP8, tag="wq")
        wv = pp.tile([128, 8 * HPC], FP8, tag="wv")
        bqk = pp.tile([64, 16], F32, tag="bqk")
        vb72 = pp.tile([128, NJ * HPC], F32, tag="vb72")
        wcv = pp.tile([128, HPC * 5 * 256], FP8, tag="wcv")
        wspe = pp.tile([128, HPC * 128], F32, tag="wspe")
        bn_s = pp.tile([128, HPC], F32, tag="bn_s")
        bn_b = pp.tile([128, HPC], F32, tag="bn_b")
        bnc_s = pp.tile([128, HPC], F32, tag="bnc_s")
        bnc_b = pp.tile([128, HPC], F32, tag="bnc_b")
        wnn = pp.tile([128, HPC * D], BF16, tag="wnn")
        bnn1 = pp.tile([128, 8], F32, tag="bnn1")
        ones8 = pp.tile([128, 32], FP8, tag="ones8")
        qk64 = pp.tile([64, 16 * N], FP8, tag="qk64")
        vpT = pp.tile([128, HPC * NJ * 128], FP8, tag="vpT")
        vcol = pp.tile([128, NJ * HPC], F32, tag="vcol")
        outT = pp.tile([128, HPC * N], BF16, tag="outT")
        spe_bc = pp.tile([128, HPC * 128], F32, tag="spe_bc")

        for t_, d_ in ((bqk, bqk_d), (wv, wv_d), (vb72, vb72_d), (bn_s, bn_s_d),
                       (bn_b, bn_b_d), (wspe, wspe_d), (bnc_s, bnc_s_d),
                       (bnc_b, bnc_b_d), (ones8, ones_d)):
            nc.sync.dma_start(t_[:], d_[:])
        P3 = 3 * PLANE
        nc.sync.dma_start(xvar[:, 0:P3], xvar_d[:, 0:P3])
        nc.sync.dma_start(xvar[:, HPC * P3:], xvar_d[:, HPC * P3:])
        nc.sync.dma_start(xq[:], xq_d[:])
        nc.sync.dma_start(wq[:], wq_d[:])
        nc.sync.dma_start(wcv[:], wcv_d[:])
        for hb in range(1, HPC):
            nc.sync.dma_start(xvar[:, hb * P3:(hb + 1) * P3],
                              xvar_d[:, hb * P3:(hb + 1) * P3])
        nc.sync.dma_start(wnn[:], wnn_d[:])
        nc.sync.dma_start(bnn1[:], bnn1_d[:])

        xqv = xq[:].rearrange("p (t n) -> p t n", t=8)
        wvv = wv[:].rearrange("p (t h) -> p t h", t=8)
        qkv_ = qk64[:].rearrange("p (b n) -> p b n", b=16)

        with tc.tile_pool(name="ptp", bufs=26) as pt_pool, \
             tc.tile_pool(name="sums", bufs=2) as sum_pool, \
             tc.tile_pool(name="spep", bufs=2) as spe_pool, \
             tc.tile_pool(name="cbrp", bufs=2) as cbr_pool, \
             tc.tile_pool(name="finp", bufs=2) as fin_pool, \
             tc.tile_pool(name="dram", bufs=1, space="DRAM") as dram_pool:
            if use_cc:
                part = dram_pool.tile([D, N], F32, name="part")
                rs0 = dram_pool.tile([CH // 2, N], F32, name="rs0")
                rs1 = dram_pool.tile([CH // 2, N], F32, name="rs1")
            else:
                part = None

            pt_tiles = {}

            def qkv_block(qps, blk, c):
                n0, nw = CHUNKS[c]
                pq = qps.tile([64, 512], F32, tag="q", name="pq")
                for t in range(4):
                    nc.tensor.matmul(
                        pq[:, 0:nw],
                        wq[:, (t * 16 + blk) * 128:(t * 16 + blk + 1) * 128]
                        .rearrange("p (i m) -> p i m", i=2),
                        xqv[:, 2 * t:2 * t + 2, n0:n0 + nw],
                        start=(t == 0), stop=(t == 3), perf_mode=DRM)
                nc.vector.tensor_scalar_add(qkv_[:, blk, n0:n0 + nw],
                                            pq[:, 0:nw], bqk[:, blk:blk + 1])

            def pd_batch(dps, c, h, p):
                n0, nw = CHUNKS[c]
                dp = dps.tile([128, 1024], F32, tag="d", name="dp")
                for i in range(2):
                    jb = 2 * p + i
                    nc.tensor.matmul(
                        dp[:, i * nw:(i + 1) * nw],
                        qkv_[:, 2 * (4 + h):2 * (4 + h) + 2,
                             jb * 128:(jb + 1) * 128],
                        qkv_[:, 2 * h:2 * h + 2, n0:n0 + nw],
                        start=True, stop=True, perf_mode=DRM)
                pt = pt_pool.tile([128, 1024], FP8, tag="pt", name="pt")
                nc.scalar.activation(pt[:, 0:2 * nw], dp[:, 0:2 * nw], AF.Exp,
                                     scale=EXPSCALE)
                pt_tiles[(c, h)].append(pt)

            def pmpo_batch(pm, po, c, h, p):
                n0, nw = CHUNKS[c]
                pt = pt_tiles[(c, h)][p]
                ptv = pt[:, 0:2 * nw].rearrange("p (i n) -> p i n", i=2)
                nc.tensor.matmul(pm[0:1, 0:nw],
                                 _ap3(ones8, 0, 16, 2, 1, 1),
                                 ptv, start=(p == 0), stop=(p == 8),
                                 perf_mode=DRM)
                nc.tensor.matmul(
                    po[:, 0:nw],
                    vpT[:, (h * NJ + 2 * p) * 128:(h * NJ + 2 * p + 2) * 128]
                    .rearrange("p (i m) -> p i m", i=2),
                    ptv, start=(p == 0), stop=(p == 8), perf_mode=DRM)

            def finish_head(pm, po, c, h):
                n0, nw = CHUNKS[c]
                rsb = sum_pool.tile([1, 512], F32, tag="rsb", name="rsb")
                with nc.allow_low_precision(reason="softmax denominators"):
                    nc.vector.reciprocal(rsb[0:1, 0:nw], pm[0:1, 0:nw])
                rbs = sum_pool.tile([128, 512], F32, tag="rbs", name="rbs")
                nc.gpsimd.partition_broadcast(rbs[:, 0:nw], rsb[0:1, 0:nw])
                nc.vector.tensor_tensor(outT[:, h * N + n0:h * N + n0 + nw],
                                        po[:, 0:nw], rbs[:, 0:nw], MULT)
                pt_tiles.pop((c, h))

            def spe_head(h):
                pcol = spe_pool.tile([128, 1], F32, tag="pcol", name="pcol")
                ctr = h * 3 * PLANE + PLANE + 48
                nc.scalar.activation(outT[:, 0:N], xvar[:, ctr:ctr + N], AFG,
                                     bias=bn_b[:, h:h + 1],
                                     scale=bn_s[:, h:h + 1], accum_out=pcol[:])
                tmp = spe_pool.tile([128, 128], F32, tag="spetmp", name="tmp")
                nc.gpsimd.tensor_scalar_mul(tmp[:],
                                            wspe[:, h * 128:(h + 1) * 128],
                                            pcol[:])
                nc.gpsimd.partition_all_reduce(
                    spe_bc[:, h * 128:(h + 1) * 128], tmp[:], 128, RADD)

            def conv_batch(cps, h, bt):
                cps = cps_ref[0]
                """3 j-blocks of the conv for head h, output directly in
                [n, c] (vpT) orientation; BN+gelu folded; v' fold on DVE."""
                hb = h * 3 * PLANE
                bias_off = HPC * 3 * PLANE
                pcT = cps.tile([128, 1024], F32, tag="d", name="pcT")
                for sub in range(3):
                    jb = bt * 3 + sub
                    for g, (o1, o2) in enumerate(CVGROUPS):
                        dy1, dx1 = OFFS[o1]
                        off1 = hb + (dx1 + 1) * PLANE + 48 + dy1 * 48 + jb * 128
                        if g < 4:
                            dy2, dx2 = OFFS[o2]
                            off2 = (hb + (dx2 + 1) * PLANE + 48 + dy2 * 48
                                    + jb * 128)
                        else:
                            off2 = bias_off
                        nc.tensor.matmul(
                            pcT[:, sub * 128:(sub + 1) * 128],
                            _ap3(xvar, off1, off2 - off1, 2, 1, 128),
                            wcv[:, (h * 5 + g) * 256:(h * 5 + g + 1) * 256]
                            .rearrange("p (i m) -> p i m", i=2),
                            start=(g == 0), stop=(g == 4), perf_mode=DRM)
                cbrT = cbr_pool.tile([128, 384], FP8, tag="cbrT", name="cbrT")
                nc.scalar.activation(cbrT[:], pcT[:, 0:384], AFG, scale=1.0 / CVS)
                for sub in range(3):
                    jb = bt * 3 + sub
                    nc.vector.scalar_tensor_tensor(
                        vpT[:, (h * NJ + jb) * 128:(h * NJ + jb + 1) * 128],
                        spe_bc[:, h * 128:(h + 1) * 128],
                        vcol[:, jb * HPC + h:jb * HPC + h + 1],
                        cbrT[:, sub * 128:(sub + 1) * 128], MULT, ADD)

            def nn1_ebo(fps, c, ebo):
                n0, nw = CHUNKS[c]
                pf = fps.tile([128, 512], F32, tag="f", name="pf")
                for h in range(HPC):
                    nc.tensor.matmul(
                        pf[:, 0:nw],
                        wnn[:, h * D + ebo * 128:h * D + (ebo + 1) * 128],
                        outT[:, h * N + n0:h * N + n0 + nw],
                        start=(h == 0), stop=(h == HPC - 1))
                fin = fin_pool.tile([128, 512], F32, tag="fin", name="fin")
                if c == 4:
                    nc.scalar.activation(fin[:, 0:nw], pf[:, 0:nw],
                                         AF.Identity, bias=bnn1[:, ebo:ebo + 1])
                else:
                    nc.vector.tensor_scalar_add(fin[:, 0:nw], pf[:, 0:nw],
                                                bnn1[:, ebo:ebo + 1])
                dst = part if use_cc else out_d
                nc.sync.dma_start(dst[ebo * 128:(ebo + 1) * 128,
                                      n0:n0 + nw], fin[:, 0:nw])

            with tc.tile_pool(name="dps", bufs=2, space="PSUM") as dps:
                # ---- WA: qkv + conv + chunk-0-head-0 dots ----
                with tc.tile_pool(name="qps", bufs=2, space="PSUM") as qps, \
                     tc.tile_pool(name="vps", bufs=1, space="PSUM") as vps:
                    cps_ref = [dps]
                    spe_head(0)                      # ACT: pooling gelu h0
                    for c in range(5):               # k blocks for head 0
                        qkv_block(qps, 8, c)
                        qkv_block(qps, 9, c)
                    qkv_block(qps, 0, 0)
                    qkv_block(qps, 1, 0)
                    pv = vps.tile([128, NJ * HPC], F32, name="pv")
                    for jb in range(NJ):
                        for t in range(4):
                            nc.tensor.matmul(
                                pv[:, jb * HPC:(jb + 1) * HPC],
                                xqv[:, 2 * t:2 * t + 2,
                                    jb * 128:(jb + 1) * 128],
                                wvv[:, 2 * t:2 * t + 2, :],
                                start=(t == 0), stop=(t == 3), perf_mode=DRM)
                    nc.vector.tensor_tensor(vcol[:], pv[:], vb72[:], ADD)
                    for h in range(1, HPC):
                        spe_head(h)
                    # conv (ACT gelus grouped) interleaved with k/q chains
                    fill = []
                    for blk in (10, 12, 14, 11, 13, 15):
                        for c in range(5):
                            fill.append((blk, c))
                    for blk in range(2, 8):
                        fill.append((blk, 0))
                    for c in range(1, 5):
                        for blk in range(0, 8):
                            fill.append((blk, c))
                    fi = 0
                    for h in range(HPC):
                        for bt in range(6):
                            conv_batch(dps, h, bt)
                            for _ in range(2):
                                if fi < len(fill):
                                    qkv_block(qps, *fill[fi])
                                    fi += 1
                    pt_tiles[(0, 0)] = []
                    for p in range(0, 9):            # exps start here
                        pd_batch(dps, 0, 0, p)
                        for _ in range(2):
                            if fi < len(fill):
                                qkv_block(qps, *fill[fi])
                                fi += 1
                    while fi < len(fill):
                        qkv_block(qps, *fill[fi])
                        fi += 1

                # ---- WB: attention pipeline + nn1 ----
                with tc.tile_pool(name="pms", bufs=1, space="PSUM") as pms, \
                     tc.tile_pool(name="pos", bufs=1, space="PSUM") as pos, \
                     tc.tile_pool(name="fps", bufs=2, space="PSUM") as fps:
                    units = [(c, h) for c in range(5) for h in range(HPC)]
                    pdlist = [(c, h, p) for (c, h) in units[1:]
                              for p in range(9)]
                    pi = 0

                    def emit_pd(n):
                        nonlocal pi
                        for _ in range(n):
                            if pi >= len(pdlist):
                                return
                            c, h, p = pdlist[pi]
                            if p == 0:
                                pt_tiles[(c, h)] = []
                            pd_batch(dps, c, h, p)
                            pi += 1

                    nnq = []

                    def emit_nn1(n):
                        for _ in range(n):
                            if nnq:
                                nn1_ebo(fps, *nnq.pop(0))

                    emit_pd(9)
                    for ui, (c, h) in enumerate(units):
                        pm = pms.tile([1, 512], F32, tag="m", name="pm")
                        po = pos.tile([128, 512], F32, tag="o", name="po")
                        for p in range(9):
                            pmpo_batch(pm, po, c, h, p)
                            emit_pd(1)
                            emit_nn1(1)
                        finish_head(pm, po, c, h)
                        if h == 3:
                            nnq.extend((c, e) for e in range(8))
                    emit_nn1(99)

                    if use_cc:
                        groups = [[0, 1], [2, 3], [4, 5], [6, 7]]
                        nc.gpsimd.collective_compute(
                            "ReduceScatter", ADD, replica_groups=groups,
                            ins=[part[0:CH, :].opt()], outs=[rs0[:].opt()])
                        nc.sync.dma_start(out_d[0:CH // 2, :], rs0[:])
                        nc.gpsimd.collective_compute(
                            "ReduceScatter", ADD, replica_groups=groups,
                            ins=[part[CH:D, :].opt()], outs=[rs1[:].opt()])
                        nc.sync.dma_start(out_d[CH // 2:CH, :], rs1[:])

    nc.compile()
    return nc


def _host_inputs(core, inp):
    b, half = core // 2, core % 2
    h0 = half * HPC
    x = np.asarray(inp["x"][b], dtype=np.float32)            # (D, N)
    Wqkv = np.asarray(inp["Wqkv"], dtype=np.float32)
    bqkv = np.asarray(inp["bqkv"], dtype=np.float32)
    Wspe = np.asarray(inp["Wspe"], dtype=np.float32)[:, :, 0, 0]   # (D, H)
    Wlocal = np.asarray(inp["Wlocal"], dtype=np.float32)     # (D, 8, 3, 3)
    Wnn1 = np.asarray(inp["Wnn1"], dtype=np.float32)
    bnn1 = np.asarray(inp["bnn1"], dtype=np.float32)

    chs = slice(h0 * HD, (h0 + HPC) * HD)

    def fold_bn(g, bta, mu, var):
        s = np.asarray(g, np.float64) / np.sqrt(np.asarray(var, np.float64) + EPS)
        return (s.astype(np.float32),
                (np.asarray(bta, np.float64) - np.asarray(mu, np.float64) * s)
                .astype(np.float32))


    # qkv moving operand: [p, dt, n]
    xq = np.ascontiguousarray(x.reshape(8, 128, N).transpose(1, 0, 2)
                              .reshape(128, 8 * N)).astype(E4)

    # conv image: raw reinterpret of x^T as (D, 48, 48); 3 dx-shifted
    # variants with 48-zero top/bottom pads, flattened per head-block
    ximg = np.ascontiguousarray(x.T).reshape(D, N)[chs]      # (512, 2304)
    xvar = np.zeros((128, HPC * 3 * PLANE + 128), np.float32)
    xv4 = xvar[:, :HPC * 3 * PLANE].reshape(128, HPC, 3, PLANE)
    img4 = ximg.reshape(HPC, 128, PS, PS)
    for hb in range(HPC):
        for vi, dx in enumerate((-1, 0, 1)):
            sh = np.zeros((128, PS, PS), np.float32)
            if dx == -1:
                sh[:, :, 1:] = img4[hb, :, :, :-1]
            elif dx == 1:
                sh[:, :, :-1] = img4[hb, :, :, 1:]
            else:
                sh = img4[hb]
            xv4[:, hb, vi, 48:48 + N] = sh.reshape(128, N)
    xvar[:, HPC * 3 * PLANE:] = 1.0 / 32.0          # conv bias plane
    xvar = np.ascontiguousarray(xvar).astype(E4)

    # q/k weights: 16 eb64 blocks (q0..q3,k0..k3 x dhalf), x8 prescale
    rows = np.concatenate(
        [np.arange(h0 * HD, (h0 + HPC) * HD) + s * D for s in range(2)])
    W8 = Wqkv[rows, :] * QS                                   # (1024, 1024)
    # wq[p, (t*16+blk)*128 + i*64 + m] = W8[blk*64+m, (2t+i)*128+p]
    wq = W8.reshape(16, 64, 4, 2, 128).transpose(4, 2, 0, 3, 1)
    wq = np.ascontiguousarray(wq.reshape(128, 64 * 128)).astype(E4)
    bqk64 = np.ascontiguousarray((QS * bqkv[rows]).reshape(16, 64).T
                                 .astype(np.float32))

    # v-sum weights [p, t, h]; bias pre-tiled [p, (jb, h)]
    vrows = np.arange(h0 * HD, (h0 + HPC) * HD) + 2 * D
    wvs = Wqkv[vrows, :].reshape(HPC, 128, D).sum(axis=1)     # (HPC, 1024)
    wvh = wvs.T.reshape(8, 128, HPC).transpose(1, 0, 2)
    wv = np.ascontiguousarray(wvh.reshape(128, 8 * HPC)).astype(E4)
    vb = bqkv[vrows].reshape(HPC, 128).sum(axis=1)
    vb72 = np.ascontiguousarray(
        np.tile(vb[None, None, :], (128, NJ, 1)).reshape(128, NJ * HPC)
        .astype(np.float32))

    # dense per-head conv weights, DoubleRow pair groups, x4 prescale
    wconv = np.zeros((HPC, 9, 128, 128), np.float32)
    for h in range(HPC):
        for co in range(128):
            g = co // 8
            cg = np.arange(g * 8, g * 8 + 8)
            for oi, (dy, dx) in enumerate(OFFS):
                wconv[h, oi, cg, co] = Wlocal[(h0 + h) * HD + co, :, dy + 1, dx + 1]
    # bn scale folded into weights; bias via the 1/32 plane (x32 here)
    bnc_s_full, bnc_b_full = fold_bn(inp["bnc_gamma"], inp["bnc_beta"],
                                     inp["bnc_mean"], inp["bnc_var"])
    sc = bnc_s_full[chs].reshape(HPC, 128)            # per (h, c)
    bc = bnc_b_full[chs].reshape(HPC, 128)
    wcv = np.zeros((128, HPC, 5, 2, 128), np.float32)
    for h in range(HPC):
        for g, (o1, o2) in enumerate(CVGROUPS):
            wcv[:, h, g, 0, :] = CVS * wconv[h, o1] * sc[h][None, :]
            if g < 4:
                wcv[:, h, g, 1, :] = CVS * wconv[h, o2] * sc[h][None, :]
            else:
                # encode 32*CVS*bc over 4 rows with residual refinement so
                # the (systematic) bias survives fp8 quantization
                S = 32.0 * CVS * bc[h]
                acc = np.zeros_like(S)
                for r in range(4):
                    step = np.asarray((S - acc) if r else S / 2, np.float32)
                    q = step.astype(E4).astype(np.float32)
                    wcv[r, h, g, 1, :] = q
                    acc += q
    wcv = np.ascontiguousarray(wcv.reshape(128, HPC * 5 * 256)).astype(E4)

    # spe block-diag matrix (folds 1/N pooling mean and attention scale)
    wspe = np.zeros((HPC, 128, 128), np.float32)
    for h in range(HPC):
        for gg in range(16):
            g = (h0 + h) * 16 + gg
            blk = Wspe[g * 8:(g + 1) * 8, :8]
            wspe[h, gg * 8:gg * 8 + 8, gg * 8:gg * 8 + 8] = blk.T
    wspe = np.ascontiguousarray(
        (wspe * (SCALE / N)).transpose(1, 0, 2).reshape(128, HPC * 128))

    bn_s, bn_b = fold_bn(inp["bn_gamma"], inp["bn_beta"], inp["bn_mean"],
                         inp["bn_var"])
    bnc_s, bnc_b = fold_bn(inp["bnc_gamma"], inp["bnc_beta"], inp["bnc_mean"],
                           inp["bnc_var"])
    shp = lambda a: np.ascontiguousarray(a[chs].reshape(HPC, 128).T)

    wnn1T = Wnn1[:, chs].T                                    # (512, 1024)
    wnn = np.ascontiguousarray(
        wnn1T.reshape(HPC, 128, D).transpose(1, 0, 2).reshape(128, HPC * D)
    ).astype(BF)
    bnn1h = np.ascontiguousarray((0.5 * bnn1).reshape(8, 128).T)

    return {
        "xq": xq, "xvar": xvar, "wq": wq, "wv": wv,
        "bqk64": bqk64, "vb72": vb72,
        "wcv": wcv, "wspe": wspe,
        "bn_s": shp(bn_s), "bn_b": shp(bn_b),
        "bnc_s": shp(bnc_s / CVS), "bnc_b": shp(bnc_b),
        "wnn": wnn, "bnn1h": bnn1h,
        "ones8": _ones_pair(),
    }


def _ones_pair():
    o = np.zeros((128, 32), np.float32)
    o[:, 0] = 1.0
    o[:, 16] = 1.0
    return o.astype(E4)


_NC = None


def kernel(**inputs):
    global _NC
    if _NC is None:
        _NC = _build()
    in_maps = [_host_inputs(c, inputs) for c in range(8)]
    res = run_bass_kernel_spmd(_NC, in_maps, core_ids=list(range(8)))
    out = np.empty((B, N, D), np.float32)
    for b in range(B):
        if USE_COLLECTIVE:
            ev, od = res.results[2 * b]["out"], res.results[2 * b + 1]["out"]
            t = np.empty((D, N), np.float32)
            t[0:256] = ev[0:256]
            t[256:512] = od[0:256]
            t[512:768] = ev[256:512]
            t[768:1024] = od[256:512]
        else:
            t = res.results[2 * b]["out"] + res.results[2 * b + 1]["out"]
        out[b] = t.T
    return out


def run_timed(**inputs):
    """Re-run with NTFF tracing to get HW exec time (best effort)."""
    global _NC
    if _NC is None:
        _NC = _build()
    in_maps = [_host_inputs(c, inputs) for c in range(8)]
    try:
        return run_bass_kernel_spmd(_NC, in_maps, core_ids=list(range(8)), trace=True)
    except Exception as e:
        print(f"trace run failed: {e}")
        return None


# revision 36
# speedup vs baseline: 1.5459x; 1.0627x over previous
"""Trainium2 Bass kernel for nn_Attention_87857851006980.

Sharding: 8 cores = 4 batches x 2 head-halves. Core c handles batch c//2,
heads [0..4) (even c) or [4..8) (odd c). Each core computes qkv for its
heads (full-d contraction), the conv/spe branches for its heads' channels,
attention for its heads, and a partial nn1 over its 512 channels; a
pair-wise ReduceScatter then sums the nn1 partials, leaving each core with
half of the output channels for its batch. Host gathers/transposes.

v2: fp8 DoubleRow matmuls for qkv / dots / softmax-sum / attn@V / conv
(2x contraction per pass), inline per-j-block v-sum (no DRAM roundtrip),
bf16 nn1, batched exp on the scalar engine, gpsimd (Pool) engine offload
for copies and partition broadcasts. Host only reshapes, quantizes dtypes,
and folds BN/bias constants.
"""
import sys
sys.path.insert(0, "/opt/trn_rl_repo")
import numpy as np
import ml_dtypes

import concourse.bacc as bacc
import concourse.bass as bass
import concourse.bass_isa as bass_isa
import concourse.tile as tile
import concourse.mybir as mybir
from concourse.bass_utils import run_bass_kernel_spmd

F32 = mybir.dt.float32
F32R = mybir.dt.float32r
BF16 = mybir.dt.bfloat16
FP8 = mybir.dt.float8e4
AF = mybir.ActivationFunctionType
ADD = mybir.AluOpType.add
MULT = mybir.AluOpType.mult
DRM = mybir.MatmulPerfMode.DoubleRow
RADD = bass_isa.ReduceOp.add
E4 = ml_dtypes.float8_e4m3
BF = ml_dtypes.bfloat16

B, D, N, H, HD = 4, 1024, 2304, 8, 128
PS = 48
HPC = 4          # heads per core
CH = HPC * HD    # 512 channels per core
NJ = 18          # key blocks of 128
EPS = 1e-5
SCALE = D ** -0.5
QS = 8.0         # q/k weight prescale (fp8 range centering)
CVS = 4.0        # conv weight prescale
EXPSCALE = SCALE / (QS * QS)
PLANE = 2400     # 48 zeros + 2304 image + 48 zeros
CHUNKS = [(0, 512), (512, 512), (1024, 512), (1536, 512), (2048, 256)]

# conv offsets (dy,dx) variant-major; 5 DoubleRow pair-groups (last
# group's second slot has zero weights and re-reads offset 8).
OFFS = [(dy, dx) for dx in (-1, 0, 1) for dy in (-1, 0, 1)]
CVGROUPS = [(0, 1), (2, 3), (4, 5), (6, 7), (8, 8)]

USE_COLLECTIVE = True


def _ap3(t, off, s1, n1, s2, n2):
    """Raw 3D AP [partitions, (s1,n1), (s2,n2)] at element offset off into
    tile t (supports overlapping / arbitrary strides)."""
    base = t[:]
    p = base.ap[0]
    return bass.AP(base.tensor, base.offset + off, [list(p), [s1, n1], [s2, n2]])


def _build(single=False, gelu=True):
    use_cc = USE_COLLECTIVE and not single
    nc = bacc.Bacc("TRN2", target_bir_lowering=False, debug=False,
                   num_devices=1 if single else 8)
    AFG = AF.Gelu if gelu else AF.Identity

    xq_d = nc.dram_tensor("xq", [128, 8 * N], FP8, kind="ExternalInput").ap()
    xvar_d = nc.dram_tensor("xvar", [128, HPC * 3 * PLANE + 128], FP8, kind="ExternalInput").ap()
    wq_d = nc.dram_tensor("wq", [128, 64 * 128], FP8, kind="ExternalInput").ap()
    wv_d = nc.dram_tensor("wv", [128, 8 * HPC], FP8, kind="ExternalInput").ap()
    bqk_d = nc.dram_tensor("bqk64", [64, 16], F32, kind="ExternalInput").ap()
    vb72_d = nc.dram_tensor("vb72", [128, NJ * HPC], F32, kind="ExternalInput").ap()
    wcv_d = nc.dram_tensor("wcv", [128, HPC * 5 * 256], FP8, kind="ExternalInput").ap()
    wspe_d = nc.dram_tensor("wspe", [128, HPC * 128], F32, kind="ExternalInput").ap()
    bn_s_d = nc.dram_tensor("bn_s", [128, HPC], F32, kind="ExternalInput").ap()
    bn_b_d = nc.dram_tensor("bn_b", [128, HPC], F32, kind="ExternalInput").ap()
    bnc_s_d = nc.dram_tensor("bnc_s", [128, HPC], F32, kind="ExternalInput").ap()
    bnc_b_d = nc.dram_tensor("bnc_b", [128, HPC], F32, kind="ExternalInput").ap()
    wnn_d = nc.dram_tensor("wnn", [128, HPC * D], BF16, kind="ExternalInput").ap()
    bnn1_d = nc.dram_tensor("bnn1h", [128, 8], F32, kind="ExternalInput").ap()
    ones_d = nc.dram_tensor("ones8", [128, 32], FP8, kind="ExternalInput").ap()
    if use_cc:
        out_d = nc.dram_tensor("out", [CH, N], F32, kind="ExternalOutput").ap()
    else:
        out_d = nc.dram_tensor("out", [D, N], F32, kind="ExternalOutput").ap()

    with tile.TileContext(nc) as tc:
      with tc.tile_pool(name="persist", bufs=1) as pp:
        xq = pp.tile([128, 8 * N], FP8, tag="xq")
        xvar = pp.tile([128, HPC * 3 * PLANE + 128], FP8, tag="xvar")
        wq = pp.tile([128, 64 * 128], FP8, tag="wq")
        wv = pp.tile([128, 8 * HPC], FP8, tag="wv")
        bqk = pp.tile([64, 16], F32, tag="bqk")
        vb72 = pp.tile([128, NJ * HPC], F32, tag="vb72")
        wcv = pp.tile([128, HPC * 5 * 256], FP8, tag="wcv")
        wspe = pp.tile([128, HPC * 128], F32, tag="wspe")
        bn_s = pp.tile([128, HPC], F32, tag="bn_s")
        bn_b = pp.tile([128, HPC], F32, tag="bn_b")
        bnc_s = pp.tile([128, HPC], F32, tag="bnc_s")
        bnc_b = pp.tile([128, HPC], F32, tag="bnc_b")
        wnn = pp.tile([128, HPC * D], BF16, tag="wnn")
        bnn1 = pp.tile([128, 8], F32, tag="bnn1")
        ones8 = pp.tile([128, 32], FP8, tag="ones8")
        qk64 = pp.tile([64, 16 * N], FP8, tag="qk64")
        vpT = pp.tile([128, HPC * NJ * 128], FP8, tag="vpT")
        vcol = pp.tile([128, NJ * HPC], F32, tag="vcol")
        outT = pp.tile([128, HPC * N], BF16, tag="outT")
        cbrS = pp.tile([128, HPC * NJ * 128], FP8, tag="cbrS")
        spe_bc = pp.tile([128, HPC * 128], F32, tag="spe_bc")

        for t_, d_ in ((bqk, bqk_d), (wv, wv_d), (vb72, vb72_d), (bn_s, bn_s_d),
                       (bn_b, bn_b_d), (wspe, wspe_d), (bnc_s, bnc_s_d),
                       (bnc_b, bnc_b_d), (ones8, ones_d)):
            nc.sync.dma_start(t_[:], d_[:])
        P3 = 3 * PLANE
        nc.sync.dma_start(xvar[:, 0:P3], xvar_d[:, 0:P3])
        nc.sync.dma_start(xvar[:, HPC * P3:], xvar_d[:, HPC * P3:])
        nc.sync.dma_start(wcv[:], wcv_d[:])
        nc.sync.dma_start(wq[:], wq_d[:])
        for hb in range(1, HPC):
            nc.sync.dma_start(xvar[:, hb * P3:(hb + 1) * P3],
                              xvar_d[:, hb * P3:(hb + 1) * P3])
        nc.sync.dma_start(xq[:], xq_d[:])
        nc.sync.dma_start(wnn[:], wnn_d[:])
        nc.sync.dma_start(bnn1[:], bnn1_d[:])

        xqv = xq[:].rearrange("p (t n) -> p t n", t=8)
        wvv = wv[:].rearrange("p (t h) -> p t h", t=8)
        qkv_ = qk64[:].rearrange("p (b n) -> p b n", b=16)

        with tc.tile_pool(name="ptp", bufs=30) as pt_pool, \
             tc.tile_pool(name="sums", bufs=2) as sum_pool, \
             tc.tile_pool(name="spep", bufs=2) as spe_pool, \
             tc.tile_pool(name="finp", bufs=2) as fin_pool, \
             tc.tile_pool(name="dram", bufs=1, space="DRAM") as dram_pool:
            if use_cc:
                part = dram_pool.tile([D, N], F32, name="part")
                rs0 = dram_pool.tile([CH // 2, N], F32, name="rs0")
                rs1 = dram_pool.tile([CH // 2, N], F32, name="rs1")
            else:
                part = None

            pt_tiles = {}

            def qkv_block(qps, blk, c):
                n0, nw = CHUNKS[c]
                pq = qps.tile([64, 512], F32, tag="q", name="pq")
                for t in range(4):
                    nc.tensor.matmul(
                        pq[:, 0:nw],
                        wq[:, (t * 16 + blk) * 128:(t * 16 + blk + 1) * 128]
                        .rearrange("p (i m) -> p i m", i=2),
                        xqv[:, 2 * t:2 * t + 2, n0:n0 + nw],
                        start=(t == 0), stop=(t == 3), perf_mode=DRM)
                nc.vector.tensor_scalar_add(qkv_[:, blk, n0:n0 + nw],
                                            pq[:, 0:nw], bqk[:, blk:blk + 1])

            def pd_batch(dps, c, h, p):
                n0, nw = CHUNKS[c]
                dp = dps.tile([128, 1024], F32, tag="d", name="dp")
                for i in range(2):
                    jb = 2 * p + i
                    nc.tensor.matmul(
                        dp[:, i * nw:(i + 1) * nw],
                        qkv_[:, 2 * (4 + h):2 * (4 + h) + 2,
                             jb * 128:(jb + 1) * 128],
                        qkv_[:, 2 * h:2 * h + 2, n0:n0 + nw],
                        start=True, stop=True, perf_mode=DRM)
                pt = pt_pool.tile([128, 1024], FP8, tag="pt", name="pt")
                nc.scalar.activation(pt[:, 0:2 * nw], dp[:, 0:2 * nw], AF.Exp,
                                     scale=EXPSCALE)
                pt_tiles[(c, h)].append(pt)

            def pmpo_batch(pm, po, c, h, p):
                n0, nw = CHUNKS[c]
                pt = pt_tiles[(c, h)][p]
                ptv = pt[:, 0:2 * nw].rearrange("p (i n) -> p i n", i=2)
                nc.tensor.matmul(pm[0:1, 0:nw],
                                 _ap3(ones8, 0, 16, 2, 1, 1),
                                 ptv, start=(p == 0), stop=(p == 8),
                                 perf_mode=DRM)
                nc.tensor.matmul(
                    po[:, 0:nw],
                    vpT[:, (h * NJ + 2 * p) * 128:(h * NJ + 2 * p + 2) * 128]
                    .rearrange("p (i m) -> p i m", i=2),
                    ptv, start=(p == 0), stop=(p == 8), perf_mode=DRM)

            def finish_head(pm, po, c, h):
                n0, nw = CHUNKS[c]
                rsb = sum_pool.tile([1, 512], F32, tag="rsb", name="rsb")
                with nc.allow_low_precision(reason="softmax denominators"):
                    nc.vector.reciprocal(rsb[0:1, 0:nw], pm[0:1, 0:nw])
                rbs = sum_pool.tile([128, 512], F32, tag="rbs", name="rbs")
                nc.gpsimd.partition_broadcast(rbs[:, 0:nw], rsb[0:1, 0:nw])
                nc.vector.tensor_tensor(outT[:, h * N + n0:h * N + n0 + nw],
                                        po[:, 0:nw], rbs[:, 0:nw], MULT)
                pt_tiles.pop((c, h))

            def spe_head(h):
                pcol = spe_pool.tile([128, 1], F32, tag="pcol", name="pcol")
                ctr = h * 3 * PLANE + PLANE + 48
                nc.scalar.activation(outT[:, 0:N], xvar[:, ctr:ctr + N], AFG,
                                     bias=bn_b[:, h:h + 1],
                                     scale=bn_s[:, h:h + 1], accum_out=pcol[:])
                tmp = spe_pool.tile([128, 128], F32, tag="spetmp", name="tmp")
                nc.gpsimd.tensor_scalar_mul(tmp[:],
                                            wspe[:, h * 128:(h + 1) * 128],
                                            pcol[:])
                nc.gpsimd.partition_all_reduce(
                    spe_bc[:, h * 128:(h + 1) * 128], tmp[:], 128, RADD)

            def conv_batch(cps, h, bt):
                cps = cps_ref[0]
                """3 j-blocks of the conv for head h, output directly in
                [n, c] (vpT) orientation; BN+gelu folded; v' fold on DVE."""
                hb = h * 3 * PLANE
                bias_off = HPC * 3 * PLANE
                pcT = cps.tile([128, 1024], F32, tag="d", name="pcT")
                for sub in range(3):
                    jb = bt * 3 + sub
                    for g, (o1, o2) in enumerate(CVGROUPS):
                        dy1, dx1 = OFFS[o1]
                        off1 = hb + (dx1 + 1) * PLANE + 48 + dy1 * 48 + jb * 128
                        if g < 4:
                            dy2, dx2 = OFFS[o2]
                            off2 = (hb + (dx2 + 1) * PLANE + 48 + dy2 * 48
                                    + jb * 128)
                        else:
                            off2 = bias_off
                        nc.tensor.matmul(
                            pcT[:, sub * 128:(sub + 1) * 128],
                            _ap3(xvar, off1, off2 - off1, 2, 1, 128),
                            wcv[:, (h * 5 + g) * 256:(h * 5 + g + 1) * 256]
                            .rearrange("p (i m) -> p i m", i=2),
                            start=(g == 0), stop=(g == 4), perf_mode=DRM)
                nc.scalar.activation(
                    cbrS[:, (h * NJ + 3 * bt) * 128:(h * NJ + 3 * bt + 3) * 128],
                    pcT[:, 0:384], AFG, scale=1.0 / CVS)

            def nn1_ebo(fps, c, ebo):
                n0, nw = CHUNKS[c]
                pf = fps.tile([128, 512], F32, tag="f", name="pf")
                for h in range(HPC):
                    nc.tensor.matmul(
                        pf[:, 0:nw],
                        wnn[:, h * D + ebo * 128:h * D + (ebo + 1) * 128],
                        outT[:, h * N + n0:h * N + n0 + nw],
                        start=(h == 0), stop=(h == HPC - 1))
                fin = fin_pool.tile([128, 512], F32, tag="fin", name="fin")
                if c == 4:
                    nc.scalar.activation(fin[:, 0:nw], pf[:, 0:nw],
                                         AF.Identity, bias=bnn1[:, ebo:ebo + 1])
                else:
                    nc.vector.tensor_scalar_add(fin[:, 0:nw], pf[:, 0:nw],
                                                bnn1[:, ebo:ebo + 1])
                dst = part if use_cc else out_d
                nc.sync.dma_start(dst[ebo * 128:(ebo + 1) * 128,
                                      n0:n0 + nw], fin[:, 0:nw])

            with tc.tile_pool(name="dps", bufs=2, space="PSUM") as dps:
                # ---- WA: qkv + conv + chunk-0-head-0 dots ----
                with tc.tile_pool(name="qps", bufs=2, space="PSUM") as qps, \
                     tc.tile_pool(name="vps", bufs=1, space="PSUM") as vps:
                    cps_ref = [dps]
                    spe_head(0)                      # ACT: pooling gelu h0
                    for c in range(5):               # k blocks for head 0
                        qkv_block(qps, 8, c)
                        qkv_block(qps, 9, c)
                    qkv_block(qps, 0, 0)
                    qkv_block(qps, 1, 0)
                    pv = vps.tile([128, NJ * HPC], F32, name="pv")
                    for jb in range(NJ):
                        for t in range(4):
                            nc.tensor.matmul(
                                pv[:, jb * HPC:(jb + 1) * HPC],
                                xqv[:, 2 * t:2 * t + 2,
                                    jb * 128:(jb + 1) * 128],
                                wvv[:, 2 * t:2 * t + 2, :],
                                start=(t == 0), stop=(t == 3), perf_mode=DRM)
                    nc.vector.tensor_tensor(vcol[:], pv[:], vb72[:], ADD)
                    # conv (ACT gelus grouped) interleaved with k/q chains
                    fill = []
                    for blk in (10, 11, 12, 13):
                        for c in range(5):
                            fill.append((blk, c))
                    for blk in range(2, 8):
                        fill.append((blk, 0))
                    for blk in range(0, 8):
                        fill.append((blk, 1))
                    for blk in (14, 15):
                        for c in range(5):
                            fill.append((blk, c))
                    for c in range(2, 5):
                        for blk in range(0, 8):
                            fill.append((blk, c))
                    fi = 0
                    for h in range(HPC):
                        if h:
                            spe_head(h)
                        for bt in range(6):
                            conv_batch(dps, h, bt)
                            for _ in range(2):
                                if fi < len(fill):
                                    qkv_block(qps, *fill[fi])
                                    fi += 1
                    pt_tiles[(0, 0)] = []
                    for p in range(0, 9):            # exps start here
                        pd_batch(dps, 0, 0, p)
                        for _ in range(2):
                            if fi < len(fill):
                                qkv_block(qps, *fill[fi])
                                fi += 1
                    while fi < len(fill):
                        qkv_block(qps, *fill[fi])
                        fi += 1
                    for h in range(HPC):             # v' folds (not in place)
                        for jb in range(NJ):
                            nc.vector.scalar_tensor_tensor(
                                vpT[:, (h * NJ + jb) * 128:
                                    (h * NJ + jb + 1) * 128],
                                spe_bc[:, h * 128:(h + 1) * 128],
                                vcol[:, jb * HPC + h:jb * HPC + h + 1],
                                cbrS[:, (h * NJ + jb) * 128:
                                     (h * NJ + jb + 1) * 128], MULT, ADD)

                # ---- WB: attention pipeline + nn1 ----
                with tc.tile_pool(name="pms", bufs=1, space="PSUM") as pms, \
                     tc.tile_pool(name="pos", bufs=1, space="PSUM") as pos, \
                     tc.tile_pool(name="fps", bufs=2, space="PSUM") as fps:
                    units = [(c, h) for c in range(5) for h in range(HPC)]
                    pdlist = [(c, h, p) for (c, h) in units[1:]
                              for p in range(9)]
                    pi = 0

                    def emit_pd(n):
                        nonlocal pi
                        for _ in range(n):
                            if pi >= len(pdlist):
                                return
                            c, h, p = pdlist[pi]
                            if p == 0:
                                pt_tiles[(c, h)] = []
                            pd_batch(dps, c, h, p)
                            pi += 1

                    nnq = []

                    def emit_nn1(n):
                        for _ in range(n):
                            if nnq:
                                nn1_ebo(fps, *nnq.pop(0))

                    emit_pd(9)
                    for ui, (c, h) in enumerate(units):
                        pm = pms.tile([1, 512], F32, tag="m", name="pm")
                        po = pos.tile([128, 512], F32, tag="o", name="po")
                        for p in range(9):
                            pmpo_batch(pm, po, c, h, p)
                            emit_pd(1)
                            emit_nn1(1)
                        finish_head(pm, po, c, h)
                        if h == 3:
                            nnq.extend((c, e) for e in range(8))
                    emit_nn1(99)

                    if use_cc:
                        groups = [[0, 1], [2, 3], [4, 5], [6, 7]]
                        nc.gpsimd.collective_compute(
                            "ReduceScatter", ADD, replica_groups=groups,
                            ins=[part[0:CH, :].opt()], outs=[rs0[:].opt()])
                        nc.sync.dma_start(out_d[0:CH // 2, :], rs0[:])
                        nc.gpsimd.collective_compute(
                            "ReduceScatter", ADD, replica_groups=groups,
                            ins=[part[CH:D, :].opt()], outs=[rs1[:].opt()])
                        nc.sync.dma_start(out_d[CH // 2:CH, :], rs1[:])

    nc.compile()
    return nc


def _host_inputs(core, inp):
    b, half = core // 2, core % 2
    h0 = half * HPC
    x = np.asarray(inp["x"][b], dtype=np.float32)            # (D, N)
    Wqkv = np.asarray(inp["Wqkv"], dtype=np.float32)
    bqkv = np.asarray(inp["bqkv"], dtype=np.float32)
    Wspe = np.asarray(inp["Wspe"], dtype=np.float32)[:, :, 0, 0]   # (D, H)
    Wlocal = np.asarray(inp["Wlocal"], dtype=np.float32)     # (D, 8, 3, 3)
    Wnn1 = np.asarray(inp["Wnn1"], dtype=np.float32)
    bnn1 = np.asarray(inp["bnn1"], dtype=np.float32)

    chs = slice(h0 * HD, (h0 + HPC) * HD)

    def fold_bn(g, bta, mu, var):
        s = np.asarray(g, np.float64) / np.sqrt(np.asarray(var, np.float64) + EPS)
        return (s.astype(np.float32),
                (np.asarray(bta, np.float64) - np.asarray(mu, np.float64) * s)
                .astype(np.float32))


    # qkv moving operand: [p, dt, n]
    xq = np.ascontiguousarray(x.reshape(8, 128, N).transpose(1, 0, 2)
                              .reshape(128, 8 * N)).astype(E4)

    # conv image: raw reinterpret of x^T as (D, 48, 48); 3 dx-shifted
    # variants with 48-zero top/bottom pads, flattened per head-block
    ximg = np.ascontiguousarray(x.T).reshape(D, N)[chs]      # (512, 2304)
    xvar = np.zeros((128, HPC * 3 * PLANE + 128), np.float32)
    xv4 = xvar[:, :HPC * 3 * PLANE].reshape(128, HPC, 3, PLANE)
    img4 = ximg.reshape(HPC, 128, PS, PS)
    for hb in range(HPC):
        for vi, dx in enumerate((-1, 0, 1)):
            sh = np.zeros((128, PS, PS), np.float32)
            if dx == -1:
                sh[:, :, 1:] = img4[hb, :, :, :-1]
            elif dx == 1:
                sh[:, :, :-1] = img4[hb, :, :, 1:]
            else:
                sh = img4[hb]
            xv4[:, hb, vi, 48:48 + N] = sh.reshape(128, N)
    xvar[:, HPC * 3 * PLANE:] = 1.0 / 32.0          # conv bias plane
    xvar = np.ascontiguousarray(xvar).astype(E4)

    # q/k weights: 16 eb64 blocks (q0..q3,k0..k3 x dhalf), x8 prescale
    rows = np.concatenate(
        [np.arange(h0 * HD, (h0 + HPC) * HD) + s * D for s in range(2)])
    W8 = Wqkv[rows, :] * QS                                   # (1024, 1024)
    # wq[p, (t*16+blk)*128 + i*64 + m] = W8[blk*64+m, (2t+i)*128+p]
    wq = W8.reshape(16, 64, 4, 2, 128).transpose(4, 2, 0, 3, 1)
    wq = np.ascontiguousarray(wq.reshape(128, 64 * 128)).astype(E4)
    bqk64 = np.ascontiguousarray((QS * bqkv[rows]).reshape(16, 64).T
                                 .astype(np.float32))

    # v-sum weights [p, t, h]; bias pre-tiled [p, (jb, h)]
    vrows = np.arange(h0 * HD, (h0 + HPC) * HD) + 2 * D
    wvs = Wqkv[vrows, :].reshape(HPC, 128, D).sum(axis=1)     # (HPC, 1024)
    wvh = wvs.T.reshape(8, 128, HPC).transpose(1, 0, 2)
    wv = np.ascontiguousarray(wvh.reshape(128, 8 * HPC)).astype(E4)
    vb = bqkv[vrows].reshape(HPC, 128).sum(axis=1)
    vb72 = np.ascontiguousarray(
        np.tile(vb[None, None, :], (128, NJ, 1)).reshape(128, NJ * HPC)
        .astype(np.float32))

    # dense per-head conv weights, DoubleRow pair groups, x4 prescale
    wconv = np.zeros((HPC, 9, 128, 128), np.float32)
    for h in range(HPC):
        for co in range(128):
            g = co // 8
            cg = np.arange(g * 8, g * 8 + 8)
            for oi, (dy, dx) in enumerate(OFFS):
                wconv[h, oi, cg, co] = Wlocal[(h0 + h) * HD + co, :, dy + 1, dx + 1]
    # bn scale folded into weights; bias via the 1/32 plane (x32 here)
    bnc_s_full, bnc_b_full = fold_bn(inp["bnc_gamma"], inp["bnc_beta"],
                                     inp["bnc_mean"], inp["bnc_var"])
    sc = bnc_s_full[chs].reshape(HPC, 128)            # per (h, c)
    bc = bnc_b_full[chs].reshape(HPC, 128)
    wcv = np.zeros((128, HPC, 5, 2, 128), np.float32)
    for h in range(HPC):
        for g, (o1, o2) in enumerate(CVGROUPS):
            wcv[:, h, g, 0, :] = CVS * wconv[h, o1] * sc[h][None, :]
            if g < 4:
                wcv[:, h, g, 1, :] = CVS * wconv[h, o2] * sc[h][None, :]
            else:
                # encode 32*CVS*bc over 4 rows with residual refinement so
                # the (systematic) bias survives fp8 quantization
                S = 32.0 * CVS * bc[h]
                acc = np.zeros_like(S)
                for r in range(4):
                    step = np.asarray((S - acc) if r else S / 2, np.float32)
                    q = step.astype(E4).astype(np.float32)
                    wcv[r, h, g, 1, :] = q
                    acc += q
    wcv = np.ascontiguousarray(wcv.reshape(128, HPC * 5 * 256)).astype(E4)

    # spe block-diag matrix (folds 1/N pooling mean and attention scale)
    wspe = np.zeros((HPC, 128, 128), np.float32)
    for h in range(HPC):
        for gg in range(16):
            g = (h0 + h) * 16 + gg
            blk = Wspe[g * 8:(g + 1) * 8, :8]
            wspe[h, gg * 8:gg * 8 + 8, gg * 8:gg * 8 + 8] = blk.T
    wspe = np.ascontiguousarray(
        (wspe * (SCALE / N)).transpose(1, 0, 2).reshape(128, HPC * 128))

    bn_s, bn_b = fold_bn(inp["bn_gamma"], inp["bn_beta"], inp["bn_mean"],
                         inp["bn_var"])
    bnc_s, bnc_b = fold_bn(inp["bnc_gamma"], inp["bnc_beta"], inp["bnc_mean"],
                           inp["bnc_var"])
    shp = lambda a: np.ascontiguousarray(a[chs].reshape(HPC, 128).T)

    wnn1T = Wnn1[:, chs].T                                    # (512, 1024)
    wnn = np.ascontiguousarray(
        wnn1T.reshape(HPC, 128, D).transpose(1, 0, 2).reshape(128, HPC * D)
    ).astype(BF)
    bnn1h = np.ascontiguousarray((0.5 * bnn1).reshape(8, 128).T)

    return {
        "xq": xq, "xvar": xvar, "wq": wq, "wv": wv,
        "bqk64": bqk64, "vb72": vb72,
        "wcv": wcv, "wspe": wspe,
        "bn_s": shp(bn_s), "bn_b": shp(bn_b),
        "bnc_s": shp(bnc_s / CVS), "bnc_b": shp(bnc_b),
        "wnn": wnn, "bnn1h": bnn1h,
        "ones8": _ones_pair(),
    }


def _ones_pair():
    o = np.zeros((128, 32), np.float32)
    o[:, 0] = 1.0
    o[:, 16] = 1.0
    return o.astype(E4)


_NC = None


def kernel(**inputs):
    global _NC
    if _NC is None:
        _NC = _build()
    in_maps = [_host_inputs(c, inputs) for c in range(8)]
    res = run_bass_kernel_spmd(_NC, in_maps, core_ids=list(range(8)))
    out = np.empty((B, N, D), np.float32)
    for b in range(B):
        if USE_COLLECTIVE:
            ev, od = res.results[2 * b]["out"], res.results[2 * b + 1]["out"]
            t = np.empty((D, N), np.float32)
            t[0:256] = ev[0:256]
            t[256:512] = od[0:256]
            t[512:768] = ev[256:512]
            t[768:1024] = od[256:512]
        else:
            t = res.results[2 * b]["out"] + res.results[2 * b + 1]["out"]
        out[b] = t.T
    return out


def run_timed(**inputs):
    """Re-run with NTFF tracing to get HW exec time (best effort)."""
    global _NC
    if _NC is None:
        _NC = _build()
    in_maps = [_host_inputs(c, inputs) for c in range(8)]
    try:
        return run_bass_kernel_spmd(_NC, in_maps, core_ids=list(range(8)), trace=True)
    except Exception as e:
        print(f"trace run failed: {e}")
        return None


# revision 40
# speedup vs baseline: 1.5759x; 1.0195x over previous
"""Trainium2 Bass kernel for nn_Attention_87857851006980.

Sharding: 8 cores = 4 batches x 2 head-halves. Core c handles batch c//2,
heads [0..4) (even c) or [4..8) (odd c). Each core computes qkv for its
heads (full-d contraction), the conv/spe branches for its heads' channels,
attention for its heads, and a partial nn1 over its 512 channels; a
pair-wise ReduceScatter then sums the nn1 partials, leaving each core with
half of the output channels for its batch. Host gathers/transposes.

v2: fp8 DoubleRow matmuls for qkv / dots / softmax-sum / attn@V / conv
(2x contraction per pass), inline per-j-block v-sum (no DRAM roundtrip),
bf16 nn1, batched exp on the scalar engine, gpsimd (Pool) engine offload
for copies and partition broadcasts. Host only reshapes, quantizes dtypes,
and folds BN/bias constants.
"""
import sys
sys.path.insert(0, "/opt/trn_rl_repo")
import numpy as np
import ml_dtypes

import concourse.bacc as bacc
import concourse.bass as bass
import concourse.bass_isa as bass_isa
import concourse.tile as tile
import concourse.mybir as mybir
from concourse.bass_utils import run_bass_kernel_spmd

F32 = mybir.dt.float32
F32R = mybir.dt.float32r
BF16 = mybir.dt.bfloat16
FP8 = mybir.dt.float8e4
AF = mybir.ActivationFunctionType
ADD = mybir.AluOpType.add
MULT = mybir.AluOpType.mult
DRM = mybir.MatmulPerfMode.DoubleRow
RADD = bass_isa.ReduceOp.add
E4 = ml_dtypes.float8_e4m3
BF = ml_dtypes.bfloat16

B, D, N, H, HD = 4, 1024, 2304, 8, 128
PS = 48
HPC = 4          # heads per core
CH = HPC * HD    # 512 channels per core
NJ = 18          # key blocks of 128
EPS = 1e-5
SCALE = D ** -0.5
QS = 8.0         # q/k weight prescale (fp8 range centering)
CVS = 4.0        # conv weight prescale
EXPSCALE = SCALE / (QS * QS)
PLANE = 2400     # 48 zeros + 2304 image + 48 zeros
CHUNKS = [(0, 512), (512, 512), (1024, 512), (1536, 512), (2048, 256)]

# conv offsets (dy,dx) variant-major; 5 DoubleRow pair-groups (last
# group's second slot has zero weights and re-reads offset 8).
OFFS = [(dy, dx) for dx in (-1, 0, 1) for dy in (-1, 0, 1)]
CVGROUPS = [(0, 1), (2, 3), (4, 5), (6, 7), (8, 8)]

USE_COLLECTIVE = True


def _ap3(t, off, s1, n1, s2, n2):
    """Raw 3D AP [partitions, (s1,n1), (s2,n2)] at element offset off into
    tile t (supports overlapping / arbitrary strides)."""
    base = t[:]
    p = base.ap[0]
    return bass.AP(base.tensor, base.offset + off, [list(p), [s1, n1], [s2, n2]])


def _build(single=False, gelu=True):
    use_cc = USE_COLLECTIVE and not single
    nc = bacc.Bacc("TRN2", target_bir_lowering=False, debug=False,
                   num_devices=1 if single else 8)
    AFG = AF.Gelu if gelu else AF.Identity

    xq_d = nc.dram_tensor("xq", [128, 8 * N], FP8, kind="ExternalInput").ap()
    xvar_d = nc.dram_tensor("xvar", [128, HPC * 3 * PLANE + 128], FP8, kind="ExternalInput").ap()
    wq_d = nc.dram_tensor("wq", [128, 64 * 128], FP8, kind="ExternalInput").ap()
    wv_d = nc.dram_tensor("wv", [128, 8 * HPC], FP8, kind="ExternalInput").ap()
    bqk_d = nc.dram_tensor("bqk64", [64, 16], F32, kind="ExternalInput").ap()
    vb72_d = nc.dram_tensor("vb72", [128, NJ * HPC], F32, kind="ExternalInput").ap()
    wcv_d = nc.dram_tensor("wcv", [128, HPC * 5 * 256], FP8, kind="ExternalInput").ap()
    wspe_d = nc.dram_tensor("wspe", [128, HPC * 128], F32, kind="ExternalInput").ap()
    bn_s_d = nc.dram_tensor("bn_s", [128, HPC], F32, kind="ExternalInput").ap()
    bn_b_d = nc.dram_tensor("bn_b", [128, HPC], F32, kind="ExternalInput").ap()
    bnc_s_d = nc.dram_tensor("bnc_s", [128, HPC], F32, kind="ExternalInput").ap()
    bnc_b_d = nc.dram_tensor("bnc_b", [128, HPC], F32, kind="ExternalInput").ap()
    wnn_d = nc.dram_tensor("wnn", [128, HPC * D], BF16, kind="ExternalInput").ap()
    bnn1_d = nc.dram_tensor("bnn1h", [128, 8], F32, kind="ExternalInput").ap()
    ones_d = nc.dram_tensor("ones8", [128, 32], FP8, kind="ExternalInput").ap()
    if use_cc:
        out_d = nc.dram_tensor("out", [CH, N], F32, kind="ExternalOutput").ap()
    else:
        out_d = nc.dram_tensor("out", [D, N], F32, kind="ExternalOutput").ap()

    with tile.TileContext(nc) as tc:
      with tc.tile_pool(name="persist", bufs=1) as pp:
        xq = pp.tile([128, 8 * N], FP8, tag="xq")
        xvar = pp.tile([128, HPC * 3 * PLANE + 128], FP8, tag="xvar")
        wq = pp.tile([128, 64 * 128], FP8, tag="wq")
        wv = pp.tile([128, 8 * HPC], FP8, tag="wv")
        bqk = pp.tile([64, 16], F32, tag="bqk")
        vb72 = pp.tile([128, NJ * HPC], F32, tag="vb72")
        wcv = pp.tile([128, HPC * 5 * 256], FP8, tag="wcv")
        wspe = pp.tile([128, HPC * 128], F32, tag="wspe")
        bn_s = pp.tile([128, HPC], F32, tag="bn_s")
        bn_b = pp.tile([128, HPC], F32, tag="bn_b")
        bnc_s = pp.tile([128, HPC], F32, tag="bnc_s")
        bnc_b = pp.tile([128, HPC], F32, tag="bnc_b")
        wnn = pp.tile([128, HPC * D], BF16, tag="wnn")
        bnn1 = pp.tile([128, 8], F32, tag="bnn1")
        ones8 = pp.tile([128, 32], FP8, tag="ones8")
        qk64 = pp.tile([64, 16 * N], FP8, tag="qk64")
        vpT = pp.tile([128, HPC * NJ * 128], FP8, tag="vpT")
        vcol = pp.tile([128, NJ * HPC], F32, tag="vcol")
        outT = pp.tile([128, HPC * N], BF16, tag="outT")
        cbrS = pp.tile([128, HPC * NJ * 128], FP8, tag="cbrS")
        spe_bc = pp.tile([128, HPC * 128], F32, tag="spe_bc")

        for t_, d_ in ((bqk, bqk_d), (wv, wv_d), (vb72, vb72_d), (bn_s, bn_s_d),
                       (bn_b, bn_b_d), (wspe, wspe_d), (bnc_s, bnc_s_d),
                       (bnc_b, bnc_b_d), (ones8, ones_d)):
            nc.sync.dma_start(t_[:], d_[:])
        P3 = 3 * PLANE
        nc.sync.dma_start(xvar[:, 0:P3], xvar_d[:, 0:P3])
        nc.sync.dma_start(xvar[:, HPC * P3:], xvar_d[:, HPC * P3:])
        nc.sync.dma_start(wcv[:], wcv_d[:])
        nc.sync.dma_start(wq[:], wq_d[:])
        for hb in range(1, HPC):
            nc.sync.dma_start(xvar[:, hb * P3:(hb + 1) * P3],
                              xvar_d[:, hb * P3:(hb + 1) * P3])
        nc.sync.dma_start(xq[:], xq_d[:])
        nc.sync.dma_start(wnn[:], wnn_d[:])
        nc.sync.dma_start(bnn1[:], bnn1_d[:])

        xqv = xq[:].rearrange("p (t n) -> p t n", t=8)
        wvv = wv[:].rearrange("p (t h) -> p t h", t=8)
        qkv_ = qk64[:].rearrange("p (b n) -> p b n", b=16)

        with tc.tile_pool(name="ptp", bufs=38) as pt_pool, \
             tc.tile_pool(name="sums", bufs=2) as sum_pool, \
             tc.tile_pool(name="spep", bufs=2) as spe_pool, \
             tc.tile_pool(name="finp", bufs=2) as fin_pool, \
             tc.tile_pool(name="dram", bufs=1, space="DRAM") as dram_pool:
            if use_cc:
                part = dram_pool.tile([D, N], F32, name="part")
                rs0 = dram_pool.tile([CH // 2, N], F32, name="rs0")
                rs1 = dram_pool.tile([CH // 2, N], F32, name="rs1")
            else:
                part = None

            pt_tiles = {}

            def qkv_block(qps, blk, c):
                n0, nw = CHUNKS[c]
                pq = qps.tile([64, 512], F32, tag="q", name="pq")
                for t in range(4):
                    nc.tensor.matmul(
                        pq[:, 0:nw],
                        wq[:, (t * 16 + blk) * 128:(t * 16 + blk + 1) * 128]
                        .rearrange("p (i m) -> p i m", i=2),
                        xqv[:, 2 * t:2 * t + 2, n0:n0 + nw],
                        start=(t == 0), stop=(t == 3), perf_mode=DRM)
                nc.vector.tensor_scalar_add(qkv_[:, blk, n0:n0 + nw],
                                            pq[:, 0:nw], bqk[:, blk:blk + 1])

            def pd_batch(dps, c, h, p):
                n0, nw = CHUNKS[c]
                dp = dps.tile([128, 1024], F32, tag="d", name="dp")
                for i in range(2):
                    jb = 2 * p + i
                    nc.tensor.matmul(
                        dp[:, i * nw:(i + 1) * nw],
                        qkv_[:, 2 * (4 + h):2 * (4 + h) + 2,
                             jb * 128:(jb + 1) * 128],
                        qkv_[:, 2 * h:2 * h + 2, n0:n0 + nw],
                        start=True, stop=True, perf_mode=DRM)
                pt = pt_pool.tile([128, 1024], FP8, tag="pt", name="pt")
                nc.scalar.activation(pt[:, 0:2 * nw], dp[:, 0:2 * nw], AF.Exp,
                                     scale=EXPSCALE)
                pt_tiles[(c, h)].append(pt)

            def pmpo_batch(pm, po, c, h, p):
                n0, nw = CHUNKS[c]
                pt = pt_tiles[(c, h)][p]
                ptv = pt[:, 0:2 * nw].rearrange("p (i n) -> p i n", i=2)
                nc.tensor.matmul(pm[0:1, 0:nw],
                                 _ap3(ones8, 0, 16, 2, 1, 1),
                                 ptv, start=(p == 0), stop=(p == 8),
                                 perf_mode=DRM)
                nc.tensor.matmul(
                    po[:, 0:nw],
                    vpT[:, (h * NJ + 2 * p) * 128:(h * NJ + 2 * p + 2) * 128]
                    .rearrange("p (i m) -> p i m", i=2),
                    ptv, start=(p == 0), stop=(p == 8), perf_mode=DRM)

            def finish_head(pm, po, c, h):
                n0, nw = CHUNKS[c]
                rsb = sum_pool.tile([1, 512], F32, tag="rsb", name="rsb")
                with nc.allow_low_precision(reason="softmax denominators"):
                    nc.vector.reciprocal(rsb[0:1, 0:nw], pm[0:1, 0:nw])
                rbs = sum_pool.tile([128, 512], F32, tag="rbs", name="rbs")
                nc.gpsimd.partition_broadcast(rbs[:, 0:nw], rsb[0:1, 0:nw])
                nc.vector.tensor_tensor(outT[:, h * N + n0:h * N + n0 + nw],
                                        po[:, 0:nw], rbs[:, 0:nw], MULT)
                pt_tiles.pop((c, h))

            def spe_head(h):
                pcol = spe_pool.tile([128, 1], F32, tag="pcol", name="pcol")
                ctr = h * 3 * PLANE + PLANE + 48
                nc.scalar.activation(outT[:, 0:N], xvar[:, ctr:ctr + N], AFG,
                                     bias=bn_b[:, h:h + 1],
                                     scale=bn_s[:, h:h + 1], accum_out=pcol[:])
                tmp = spe_pool.tile([128, 128], F32, tag="spetmp", name="tmp")
                nc.gpsimd.tensor_scalar_mul(tmp[:],
                                            wspe[:, h * 128:(h + 1) * 128],
                                            pcol[:])
                nc.gpsimd.partition_all_reduce(
                    spe_bc[:, h * 128:(h + 1) * 128], tmp[:], 128, RADD)

            def conv_batch(cps, h, bt):
                cps = cps_ref[0]
                """3 j-blocks of the conv for head h, output directly in
                [n, c] (vpT) orientation; BN+gelu folded; v' fold on DVE."""
                hb = h * 3 * PLANE
                bias_off = HPC * 3 * PLANE
                pcT = cps.tile([128, 1024], F32, tag="d", name="pcT")
                for sub in range(3):
                    jb = bt * 3 + sub
                    for g, (o1, o2) in enumerate(CVGROUPS):
                        dy1, dx1 = OFFS[o1]
                        off1 = hb + (dx1 + 1) * PLANE + 48 + dy1 * 48 + jb * 128
                        if g < 4:
                            dy2, dx2 = OFFS[o2]
                            off2 = (hb + (dx2 + 1) * PLANE + 48 + dy2 * 48
                                    + jb * 128)
                        else:
                            off2 = bias_off
                        nc.tensor.matmul(
                            pcT[:, sub * 128:(sub + 1) * 128],
                            _ap3(xvar, off1, off2 - off1, 2, 1, 128),
                            wcv[:, (h * 5 + g) * 256:(h * 5 + g + 1) * 256]
                            .rearrange("p (i m) -> p i m", i=2),
                            start=(g == 0), stop=(g == 4), perf_mode=DRM)
                nc.scalar.activation(
                    cbrS[:, (h * NJ + 3 * bt) * 128:(h * NJ + 3 * bt + 3) * 128],
                    pcT[:, 0:384], AFG, scale=1.0 / CVS)

            def nn1_ebo(fps, c, ebo):
                n0, nw = CHUNKS[c]
                pf = fps.tile([128, 512], F32, tag="f", name="pf")
                for h in range(HPC):
                    nc.tensor.matmul(
                        pf[:, 0:nw],
                        wnn[:, h * D + ebo * 128:h * D + (ebo + 1) * 128],
                        outT[:, h * N + n0:h * N + n0 + nw],
                        start=(h == 0), stop=(h == HPC - 1))
                fin = fin_pool.tile([128, 512], F32, tag="fin", name="fin")
                if c == 4:
                    nc.scalar.activation(fin[:, 0:nw], pf[:, 0:nw],
                                         AF.Identity, bias=bnn1[:, ebo:ebo + 1])
                else:
                    nc.vector.tensor_scalar_add(fin[:, 0:nw], pf[:, 0:nw],
                                                bnn1[:, ebo:ebo + 1])
                dst = part if use_cc else out_d
                nc.sync.dma_start(dst[ebo * 128:(ebo + 1) * 128,
                                      n0:n0 + nw], fin[:, 0:nw])

            with tc.tile_pool(name="dps", bufs=2, space="PSUM") as dps:
                # ---- WA: qkv + conv + chunk-0-head-0 dots ----
                with tc.tile_pool(name="qps", bufs=2, space="PSUM") as qps, \
                     tc.tile_pool(name="vps", bufs=1, space="PSUM") as vps:
                    cps_ref = [dps]
                    spe_head(0)                      # ACT: pooling gelu h0
                    for c in range(5):               # k blocks for head 0
                        qkv_block(qps, 8, c)
                        qkv_block(qps, 9, c)
                    qkv_block(qps, 0, 0)
                    qkv_block(qps, 1, 0)
                    pv = vps.tile([128, NJ * HPC], F32, name="pv")
                    for jb in range(NJ):
                        for t in range(4):
                            nc.tensor.matmul(
                                pv[:, jb * HPC:(jb + 1) * HPC],
                                xqv[:, 2 * t:2 * t + 2,
                                    jb * 128:(jb + 1) * 128],
                                wvv[:, 2 * t:2 * t + 2, :],
                                start=(t == 0), stop=(t == 3), perf_mode=DRM)
                    nc.vector.tensor_tensor(vcol[:], pv[:], vb72[:], ADD)
                    # conv (ACT gelus grouped) interleaved with k/q chains
                    fill = []
                    for blk in (10, 11, 12, 13):
                        for c in range(5):
                            fill.append((blk, c))
                    for blk in range(2, 8):
                        fill.append((blk, 0))
                    for blk in range(0, 8):
                        fill.append((blk, 1))
                    for blk in (14, 15):
                        for c in range(5):
                            fill.append((blk, c))
                    for c in range(2, 5):
                        for blk in range(0, 8):
                            fill.append((blk, c))
                    fi = 0
                    for h in range(HPC):
                        if h:
                            spe_head(h)
                        for bt in range(6):
                            conv_batch(dps, h, bt)
                            for _ in range(2):
                                if fi < len(fill):
                                    qkv_block(qps, *fill[fi])
                                    fi += 1
                    pt_tiles[(0, 0)] = []
                    for p in range(0, 9):            # exps start here
                        pd_batch(dps, 0, 0, p)
                        for _ in range(2):
                            if fi < len(fill):
                                qkv_block(qps, *fill[fi])
                                fi += 1
                    while fi < len(fill):
                        qkv_block(qps, *fill[fi])
                        fi += 1
                    for h in range(HPC):             # v' folds (not in place)
                        for jb in range(NJ):
                            nc.vector.scalar_tensor_tensor(
                                vpT[:, (h * NJ + jb) * 128:
                                    (h * NJ + jb + 1) * 128],
                                spe_bc[:, h * 128:(h + 1) * 128],
                                vcol[:, jb * HPC + h:jb * HPC + h + 1],
                                cbrS[:, (h * NJ + jb) * 128:
                                     (h * NJ + jb + 1) * 128], MULT, ADD)

                # ---- WB: attention pipeline + nn1 ----
                with tc.tile_pool(name="pms", bufs=1, space="PSUM") as pms, \
                     tc.tile_pool(name="pos", bufs=1, space="PSUM") as pos, \
                     tc.tile_pool(name="fps", bufs=2, space="PSUM") as fps:
                    units = [(c, h) for c in range(5) for h in range(HPC)]
                    pdlist = [(c, h, p) for (c, h) in units[1:]
                              for p in range(9)]
                    pi = 0

                    def emit_pd(n):
                        nonlocal pi
                        for _ in range(n):
                            if pi >= len(pdlist):
                                return
                            c, h, p = pdlist[pi]
                            if p == 0:
                                pt_tiles[(c, h)] = []
                            pd_batch(dps, c, h, p)
                            pi += 1

                    nnq = []

                    def emit_nn1(n):
                        for _ in range(n):
                            if nnq:
                                nn1_ebo(fps, *nnq.pop(0))

                    emit_pd(9)
                    for ui, (c, h) in enumerate(units):
                        pm = pms.tile([1, 512], F32, tag="m", name="pm")
                        po = pos.tile([128, 512], F32, tag="o", name="po")
                        for p in range(9):
                            pmpo_batch(pm, po, c, h, p)
                            emit_pd(1)
                            emit_nn1(1)
                        finish_head(pm, po, c, h)
                        if h == 3:
                            nnq.extend((c, e) for e in range(8))
                    emit_nn1(99)

                    if use_cc:
                        groups = [[0, 1], [2, 3], [4, 5], [6, 7]]
                        nc.gpsimd.collective_compute(
                            "ReduceScatter", ADD, replica_groups=groups,
                            ins=[part[0:CH, :].opt()], outs=[rs0[:].opt()])
                        nc.sync.dma_start(out_d[0:CH // 2, :], rs0[:])
                        nc.gpsimd.collective_compute(
                            "ReduceScatter", ADD, replica_groups=groups,
                            ins=[part[CH:D, :].opt()], outs=[rs1[:].opt()])
                        nc.sync.dma_start(out_d[CH // 2:CH, :], rs1[:])

    nc.compile()
    return nc


def _host_inputs(core, inp):
    b, half = core // 2, core % 2
    h0 = half * HPC
    x = np.asarray(inp["x"][b], dtype=np.float32)            # (D, N)
    Wqkv = np.asarray(inp["Wqkv"], dtype=np.float32)
    bqkv = np.asarray(inp["bqkv"], dtype=np.float32)
    Wspe = np.asarray(inp["Wspe"], dtype=np.float32)[:, :, 0, 0]   # (D, H)
    Wlocal = np.asarray(inp["Wlocal"], dtype=np.float32)     # (D, 8, 3, 3)
    Wnn1 = np.asarray(inp["Wnn1"], dtype=np.float32)
    bnn1 = np.asarray(inp["bnn1"], dtype=np.float32)

    chs = slice(h0 * HD, (h0 + HPC) * HD)

    def fold_bn(g, bta, mu, var):
        s = np.asarray(g, np.float64) / np.sqrt(np.asarray(var, np.float64) + EPS)
        return (s.astype(np.float32),
                (np.asarray(bta, np.float64) - np.asarray(mu, np.float64) * s)
                .astype(np.float32))


    # qkv moving operand: [p, dt, n]
    xq = np.ascontiguousarray(x.reshape(8, 128, N).transpose(1, 0, 2)
                              .reshape(128, 8 * N)).astype(E4)

    # conv image: raw reinterpret of x^T as (D, 48, 48); 3 dx-shifted
    # variants with 48-zero top/bottom pads, flattened per head-block
    ximg = np.ascontiguousarray(x.T).reshape(D, N)[chs]      # (512, 2304)
    xvar = np.zeros((128, HPC * 3 * PLANE + 128), np.float32)
    xv4 = xvar[:, :HPC * 3 * PLANE].reshape(128, HPC, 3, PLANE)
    img4 = ximg.reshape(HPC, 128, PS, PS)
    for hb in range(HPC):
        for vi, dx in enumerate((-1, 0, 1)):
            sh = np.zeros((128, PS, PS), np.float32)
            if dx == -1:
                sh[:, :, 1:] = img4[hb, :, :, :-1]
            elif dx == 1:
                sh[:, :, :-1] = img4[hb, :, :, 1:]
            else:
                sh = img4[hb]
            xv4[:, hb, vi, 48:48 + N] = sh.reshape(128, N)
    xvar[:, HPC * 3 * PLANE:] = 1.0 / 32.0          # conv bias plane
    xvar = np.ascontiguousarray(xvar).astype(E4)

    # q/k weights: 16 eb64 blocks (q0..q3,k0..k3 x dhalf), x8 prescale
    rows = np.concatenate(
        [np.arange(h0 * HD, (h0 + HPC) * HD) + s * D for s in range(2)])
    W8 = Wqkv[rows, :] * QS                                   # (1024, 1024)
    # wq[p, (t*16+blk)*128 + i*64 + m] = W8[blk*64+m, (2t+i)*128+p]
    wq = W8.reshape(16, 64, 4, 2, 128).transpose(4, 2, 0, 3, 1)
    wq = np.ascontiguousarray(wq.reshape(128, 64 * 128)).astype(E4)
    bqk64 = np.ascontiguousarray((QS * bqkv[rows]).reshape(16, 64).T
                                 .astype(np.float32))

    # v-sum weights [p, t, h]; bias pre-tiled [p, (jb, h)]
    vrows = np.arange(h0 * HD, (h0 + HPC) * HD) + 2 * D
    wvs = Wqkv[vrows, :].reshape(HPC, 128, D).sum(axis=1)     # (HPC, 1024)
    wvh = wvs.T.reshape(8, 128, HPC).transpose(1, 0, 2)
    wv = np.ascontiguousarray(wvh.reshape(128, 8 * HPC)).astype(E4)
    vb = bqkv[vrows].reshape(HPC, 128).sum(axis=1)
    vb72 = np.ascontiguousarray(
        np.tile(vb[None, None, :], (128, NJ, 1)).reshape(128, NJ * HPC)
        .astype(np.float32))

    # dense per-head conv weights, DoubleRow pair groups, x4 prescale
    wconv = np.zeros((HPC, 9, 128, 128), np.float32)
    for h in range(HPC):
        for co in range(128):
            g = co // 8
            cg = np.arange(g * 8, g * 8 + 8)
            for oi, (dy, dx) in enumerate(OFFS):
                wconv[h, oi, cg, co] = Wlocal[(h0 + h) * HD + co, :, dy + 1, dx + 1]
    # bn scale folded into weights; bias via the 1/32 plane (x32 here)
    bnc_s_full, bnc_b_full = fold_bn(inp["bnc_gamma"], inp["bnc_beta"],
                                     inp["bnc_mean"], inp["bnc_var"])
    sc = bnc_s_full[chs].reshape(HPC, 128)            # per (h, c)
    bc = bnc_b_full[chs].reshape(HPC, 128)
    wcv = np.zeros((128, HPC, 5, 2, 128), np.float32)
    for h in range(HPC):
        for g, (o1, o2) in enumerate(CVGROUPS):
            wcv[:, h, g, 0, :] = CVS * wconv[h, o1] * sc[h][None, :]
            if g < 4:
                wcv[:, h, g, 1, :] = CVS * wconv[h, o2] * sc[h][None, :]
            else:
                # encode 32*CVS*bc over 4 rows with residual refinement so
                # the (systematic) bias survives fp8 quantization
                S = 32.0 * CVS * bc[h]
                acc = np.zeros_like(S)
                for r in range(4):
                    step = np.asarray((S - acc) if r else S / 2, np.float32)
                    q = step.astype(E4).astype(np.float32)
                    wcv[r, h, g, 1, :] = q
                    acc += q
    wcv = np.ascontiguousarray(wcv.reshape(128, HPC * 5 * 256)).astype(E4)

    # spe block-diag matrix (folds 1/N pooling mean and attention scale)
    wspe = np.zeros((HPC, 128, 128), np.float32)
    for h in range(HPC):
        for gg in range(16):
            g = (h0 + h) * 16 + gg
            blk = Wspe[g * 8:(g + 1) * 8, :8]
            wspe[h, gg * 8:gg * 8 + 8, gg * 8:gg * 8 + 8] = blk.T
    wspe = np.ascontiguousarray(
        (wspe * (SCALE / N)).transpose(1, 0, 2).reshape(128, HPC * 128))

    bn_s, bn_b = fold_bn(inp["bn_gamma"], inp["bn_beta"], inp["bn_mean"],
                         inp["bn_var"])
    bnc_s, bnc_b = fold_bn(inp["bnc_gamma"], inp["bnc_beta"], inp["bnc_mean"],
                           inp["bnc_var"])
    shp = lambda a: np.ascontiguousarray(a[chs].reshape(HPC, 128).T)

    wnn1T = Wnn1[:, chs].T                                    # (512, 1024)
    wnn = np.ascontiguousarray(
        wnn1T.reshape(HPC, 128, D).transpose(1, 0, 2).reshape(128, HPC * D)
    ).astype(BF)
    bnn1h = np.ascontiguousarray((0.5 * bnn1).reshape(8, 128).T)

    return {
        "xq": xq, "xvar": xvar, "wq": wq, "wv": wv,
        "bqk64": bqk64, "vb72": vb72,
        "wcv": wcv, "wspe": wspe,
        "bn_s": shp(bn_s), "bn_b": shp(bn_b),
        "bnc_s": shp(bnc_s / CVS), "bnc_b": shp(bnc_b),
        "wnn": wnn, "bnn1h": bnn1h,
        "ones8": _ones_pair(),
    }


def _ones_pair():
    o = np.zeros((128, 32), np.float32)
    o[:, 0] = 1.0
    o[:, 16] = 1.0
    return o.astype(E4)


_NC = None


def kernel(**inputs):
    global _NC
    if _NC is None:
        _NC = _build()
    in_maps = [_host_inputs(c, inputs) for c in range(8)]
    res = run_bass_kernel_spmd(_NC, in_maps, core_ids=list(range(8)))
    out = np.empty((B, N, D), np.float32)
    for b in range(B):
        if USE_COLLECTIVE:
            ev, od = res.results[2 * b]["out"], res.results[2 * b + 1]["out"]
            t = np.empty((D, N), np.float32)
            t[0:256] = ev[0:256]
            t[256:512] = od[0:256]
            t[512:768] = ev[256:512]
            t[768:1024] = od[256:512]
        else:
            t = res.results[2 * b]["out"] + res.results[2 * b + 1]["out"]
        out[b] = t.T
    return out


def run_timed(**inputs):
    """Re-run with NTFF tracing to get HW exec time (best effort)."""
    global _NC
    if _NC is None:
        _NC = _build()
    in_maps = [_host_inputs(c, inputs) for c in range(8)]
    try:
        return run_bass_kernel_spmd(_NC, in_maps, core_ids=list(range(8)), trace=True)
    except Exception as e:
        print(f"trace run failed: {e}")
        return None


# revision 48
# speedup vs baseline: 1.6624x; 1.0549x over previous
"""Trainium2 Bass kernel for nn_Attention_87857851006980.

Sharding: 8 cores = 4 batches x 2 head-halves. Core c handles batch c//2,
heads [0..4) (even c) or [4..8) (odd c). Each core computes qkv for its
heads (full-d contraction), the conv/spe branches for its heads' channels,
attention for its heads, and a partial nn1 over its 512 channels; a
pair-wise ReduceScatter then sums the nn1 partials, leaving each core with
half of the output channels for its batch. Host gathers/transposes.

v2: fp8 DoubleRow matmuls for qkv / dots / softmax-sum / attn@V / conv
(2x contraction per pass), inline per-j-block v-sum (no DRAM roundtrip),
bf16 nn1, batched exp on the scalar engine, gpsimd (Pool) engine offload
for copies and partition broadcasts. Host only reshapes, quantizes dtypes,
and folds BN/bias constants.
"""
import sys
sys.path.insert(0, "/opt/trn_rl_repo")
import numpy as np
import ml_dtypes

import concourse.bacc as bacc
import concourse.bass as bass
import concourse.bass_isa as bass_isa
import concourse.tile as tile
import concourse.mybir as mybir
from concourse.bass_utils import run_bass_kernel_spmd

F32 = mybir.dt.float32
F32R = mybir.dt.float32r
BF16 = mybir.dt.bfloat16
FP8 = mybir.dt.float8e4
AF = mybir.ActivationFunctionType
ADD = mybir.AluOpType.add
MULT = mybir.AluOpType.mult
DRM = mybir.MatmulPerfMode.DoubleRow
RADD = bass_isa.ReduceOp.add
E4 = ml_dtypes.float8_e4m3
BF = ml_dtypes.bfloat16

B, D, N, H, HD = 4, 1024, 2304, 8, 128
PS = 48
HPC = 4          # heads per core
CH = HPC * HD    # 512 channels per core
NJ = 18          # key blocks of 128
EPS = 1e-5
SCALE = D ** -0.5
QS = 8.0         # q/k weight prescale (fp8 range centering)
CVS = 4.0        # conv weight prescale
EXPSCALE = SCALE / (QS * QS)
PLANE = 2400     # 48 zeros + 2304 image + 48 zeros
CHUNKS = [(0, 512), (512, 512), (1024, 512), (1536, 512), (2048, 256)]

# conv offsets (dy,dx) variant-major; 5 DoubleRow pair-groups (last
# group's second slot has zero weights and re-reads offset 8).
OFFS = [(dy, dx) for dx in (-1, 0, 1) for dy in (-1, 0, 1)]
CVGROUPS = [(0, 1), (2, 3), (4, 5), (6, 7), (8, 8)]

USE_COLLECTIVE = True


def _ap3(t, off, s1, n1, s2, n2):
    """Raw 3D AP [partitions, (s1,n1), (s2,n2)] at element offset off into
    tile t (supports overlapping / arbitrary strides)."""
    base = t[:]
    p = base.ap[0]
    return bass.AP(base.tensor, base.offset + off, [list(p), [s1, n1], [s2, n2]])


def _build(single=False, gelu=True):
    use_cc = USE_COLLECTIVE and not single
    nc = bacc.Bacc("TRN2", target_bir_lowering=False, debug=False,
                   num_devices=1 if single else 8)
    AFG = AF.Gelu if gelu else AF.Identity

    xq_d = nc.dram_tensor("xq", [128, 8 * N], FP8, kind="ExternalInput").ap()
    xvar_d = nc.dram_tensor("xvar", [128, HPC * 3 * PLANE + 128], FP8, kind="ExternalInput").ap()
    wq_d = nc.dram_tensor("wq", [128, 64 * 128], FP8, kind="ExternalInput").ap()
    bqk_d = nc.dram_tensor("bqk64", [64, 16], F32, kind="ExternalInput").ap()
    cf_d = nc.dram_tensor("cf32", [128, 600], F32, kind="ExternalInput").ap()
    c8_d = nc.dram_tensor("c8", [128, 64], FP8, kind="ExternalInput").ap()
    wcv_d = nc.dram_tensor("wcv", [128, HPC * 5 * 256], FP8, kind="ExternalInput").ap()
    wnn_d = nc.dram_tensor("wnn", [128, HPC * D], BF16, kind="ExternalInput").ap()
    bnn1_d = nc.dram_tensor("bnn1h", [128, 8], F32, kind="ExternalInput").ap()
    if use_cc:
        out_d = nc.dram_tensor("out", [CH, N], F32, kind="ExternalOutput").ap()
    else:
        out_d = nc.dram_tensor("out", [D, N], F32, kind="ExternalOutput").ap()

    with tile.TileContext(nc) as tc:
      with tc.tile_pool(name="persist", bufs=1) as pp:
        xq = pp.tile([128, 8 * N], FP8, tag="xq")
        xvar = pp.tile([128, HPC * 3 * PLANE + 128], FP8, tag="xvar")
        wq = pp.tile([128, 64 * 128], FP8, tag="wq")
        bqk = pp.tile([64, 16], F32, tag="bqk")
        cf32 = pp.tile([128, 600], F32, tag="cf32")
        c8t = pp.tile([128, 64], FP8, tag="c8t")
        wcv = pp.tile([128, HPC * 5 * 256], FP8, tag="wcv")
        bn_s = cf32[:][:, 0:4]
        bn_b = cf32[:][:, 4:8]
        vb72 = cf32[:][:, 8:80]
        wspe = cf32[:][:, 80:592]
        bnc_s = cf32[:][:, 592:596]
        bnc_b = cf32[:][:, 596:600]
        wv = c8t[:][:, 0:32]
        ones8 = c8t
        wnn = pp.tile([128, HPC * D], BF16, tag="wnn")
        bnn1 = pp.tile([128, 8], F32, tag="bnn1")
        qk64 = pp.tile([64, 16 * N], FP8, tag="qk64")
        vpT = pp.tile([128, HPC * NJ * 128], FP8, tag="vpT")
        vcol = pp.tile([128, NJ * HPC], F32, tag="vcol")
        outT = pp.tile([128, HPC * N], BF16, tag="outT")
        cbrS = pp.tile([128, HPC * NJ * 128], FP8, tag="cbrS")
        spe_bc = pp.tile([128, HPC * 128], F32, tag="spe_bc")

        P3 = 3 * PLANE
        nc.sync.dma_start(cf32[:], cf_d[:])
        nc.sync.dma_start(xvar[:, 0:P3], xvar_d[:, 0:P3])
        nc.sync.dma_start(bqk[:], bqk_d[:])
        nc.sync.dma_start(c8t[:], c8_d[:])
        nc.sync.dma_start(xvar[:, HPC * P3:], xvar_d[:, HPC * P3:])
        nc.sync.dma_start(wcv[:], wcv_d[:])
        for hb in range(1, HPC):
            nc.sync.dma_start(xvar[:, hb * P3:(hb + 1) * P3],
                              xvar_d[:, hb * P3:(hb + 1) * P3])
        nc.sync.dma_start(wq[:], wq_d[:])
        nc.sync.dma_start(xq[:], xq_d[:])
        nc.sync.dma_start(wnn[:], wnn_d[:])
        nc.sync.dma_start(bnn1[:], bnn1_d[:])

        xqv = xq[:].rearrange("p (t n) -> p t n", t=8)
        wvv = wv.rearrange("p (t h) -> p t h", t=8)
        qkv_ = qk64[:].rearrange("p (b n) -> p b n", b=16)

        with tc.tile_pool(name="ptp", bufs=38) as pt_pool, \
             tc.tile_pool(name="sums", bufs=2) as sum_pool, \
             tc.tile_pool(name="spep", bufs=2) as spe_pool, \
             tc.tile_pool(name="finp", bufs=2) as fin_pool, \
             tc.tile_pool(name="dram", bufs=1, space="DRAM") as dram_pool:
            if use_cc:
                part = dram_pool.tile([D, N], F32, name="part")
                rs0 = dram_pool.tile([CH // 2, N], F32, name="rs0")
                rs1 = dram_pool.tile([CH // 2, N], F32, name="rs1")
            else:
                part = None

            pt_tiles = {}

            def qkv_block(qps, blk, c):
                n0, nw = CHUNKS[c]
                pq = qps.tile([64, 512], F32, tag="q", name="pq")
                for t in range(4):
                    nc.tensor.matmul(
                        pq[:, 0:nw],
                        wq[:, (t * 16 + blk) * 128:(t * 16 + blk + 1) * 128]
                        .rearrange("p (i m) -> p i m", i=2),
                        xqv[:, 2 * t:2 * t + 2, n0:n0 + nw],
                        start=(t == 0), stop=(t == 3), perf_mode=DRM)
                nc.vector.tensor_scalar_add(qkv_[:, blk, n0:n0 + nw],
                                            pq[:, 0:nw], bqk[:, blk:blk + 1])

            def pd_batch(dps, c, h, p):
                n0, nw = CHUNKS[c]
                if nw < 512 and p % 2:
                    return                       # folded into the even batch
                njb = 2 if nw == 512 else (2 if p == 8 else 4)
                dp = dps.tile([128, 1024], F32, tag="d", name="dp")
                for i in range(njb):
                    jb = 2 * p + i
                    nc.tensor.matmul(
                        dp[:, i * nw:(i + 1) * nw],
                        qkv_[:, 2 * (4 + h):2 * (4 + h) + 2,
                             jb * 128:(jb + 1) * 128],
                        qkv_[:, 2 * h:2 * h + 2, n0:n0 + nw],
                        start=True, stop=True, perf_mode=DRM)
                pt = pt_pool.tile([128, 1024], FP8, tag="pt", name="pt")
                nc.scalar.activation(pt[:, 0:njb * nw], dp[:, 0:njb * nw],
                                     AF.Exp, scale=EXPSCALE)
                pt_tiles[(c, h)].append(pt)

            def pmpo_batch(pm, po, c, h, p):
                n0, nw = CHUNKS[c]
                if nw == 512:
                    pt = pt_tiles[(c, h)][p]
                    ptv = pt[:, 0:2 * nw].rearrange("p (i n) -> p i n", i=2)
                else:
                    pt = pt_tiles[(c, h)][p // 2]
                    off = (p % 2) * 2 * nw
                    ptv = pt[:, off:off + 2 * nw].rearrange(
                        "p (i n) -> p i n", i=2)
                nc.tensor.matmul(pm[0:1, 0:nw],
                                 _ap3(ones8, 32, 16, 2, 1, 1),
                                 ptv, start=(p == 0), stop=(p == 8),
                                 perf_mode=DRM)
                nc.tensor.matmul(
                    po[:, 0:nw],
                    vpT[:, (h * NJ + 2 * p) * 128:(h * NJ + 2 * p + 2) * 128]
                    .rearrange("p (i m) -> p i m", i=2),
                    ptv, start=(p == 0), stop=(p == 8), perf_mode=DRM)

            def finish_head(pm, po, c, h):
                n0, nw = CHUNKS[c]
                rsb = sum_pool.tile([1, 512], F32, tag="rsb", name="rsb")
                with nc.allow_low_precision(reason="softmax denominators"):
                    nc.vector.reciprocal(rsb[0:1, 0:nw], pm[0:1, 0:nw])
                rbs = sum_pool.tile([128, 512], F32, tag="rbs", name="rbs")
                nc.gpsimd.partition_broadcast(rbs[:, 0:nw], rsb[0:1, 0:nw])
                nc.vector.tensor_tensor(outT[:, h * N + n0:h * N + n0 + nw],
                                        po[:, 0:nw], rbs[:, 0:nw], MULT)
                pt_tiles.pop((c, h))

            def spe_head(h):
                pcol = spe_pool.tile([128, 1], F32, tag="pcol", name="pcol")
                ctr = h * 3 * PLANE + PLANE + 48
                nc.scalar.activation(outT[:, 0:N], xvar[:, ctr:ctr + N], AFG,
                                     bias=bn_b[:, h:h + 1],
                                     scale=bn_s[:, h:h + 1], accum_out=pcol[:])
                tmp = spe_pool.tile([128, 128], F32, tag="spetmp", name="tmp")
                nc.gpsimd.tensor_scalar_mul(tmp[:],
                                            wspe[:, h * 128:(h + 1) * 128],
                                            pcol[:])
                nc.gpsimd.partition_all_reduce(
                    spe_bc[:, h * 128:(h + 1) * 128], tmp[:], 128, RADD)

            def conv_batch(cps, h, bt):
                cps = cps_ref[0]
                """3 j-blocks of the conv for head h, output directly in
                [n, c] (vpT) orientation; BN+gelu folded; v' fold on DVE."""
                hb = h * 3 * PLANE
                bias_off = HPC * 3 * PLANE
                pcT = cps.tile([128, 1024], F32, tag="d", name="pcT")
                for sub in range(3):
                    jb = bt * 3 + sub
                    for g, (o1, o2) in enumerate(CVGROUPS):
                        dy1, dx1 = OFFS[o1]
                        off1 = hb + (dx1 + 1) * PLANE + 48 + dy1 * 48 + jb * 128
                        if g < 4:
                            dy2, dx2 = OFFS[o2]
                            off2 = (hb + (dx2 + 1) * PLANE + 48 + dy2 * 48
                                    + jb * 128)
                        else:
                            off2 = bias_off
                        nc.tensor.matmul(
                            pcT[:, sub * 128:(sub + 1) * 128],
                            _ap3(xvar, off1, off2 - off1, 2, 1, 128),
                            wcv[:, (h * 5 + g) * 256:(h * 5 + g + 1) * 256]
                            .rearrange("p (i m) -> p i m", i=2),
                            start=(g == 0), stop=(g == 4), perf_mode=DRM)
                nc.scalar.activation(
                    cbrS[:, (h * NJ + 3 * bt) * 128:(h * NJ + 3 * bt + 3) * 128],
                    pcT[:, 0:384], AFG, scale=1.0 / CVS)

            def nn1_ebo(fps, c, ebo):
                n0, nw = CHUNKS[c]
                pf = fps.tile([128, 512], F32, tag="f", name="pf")
                for h in range(HPC):
                    nc.tensor.matmul(
                        pf[:, 0:nw],
                        wnn[:, h * D + ebo * 128:h * D + (ebo + 1) * 128],
                        outT[:, h * N + n0:h * N + n0 + nw],
                        start=(h == 0), stop=(h == HPC - 1))
                fin = fin_pool.tile([128, 512], F32, tag="fin", name="fin")
                if c == 4:
                    nc.scalar.activation(fin[:, 0:nw], pf[:, 0:nw],
                                         AF.Identity, bias=bnn1[:, ebo:ebo + 1])
                else:
                    nc.vector.tensor_scalar_add(fin[:, 0:nw], pf[:, 0:nw],
                                                bnn1[:, ebo:ebo + 1])
                dst = part if use_cc else out_d
                nc.sync.dma_start(dst[ebo * 128:(ebo + 1) * 128,
                                      n0:n0 + nw], fin[:, 0:nw])

            with tc.tile_pool(name="dps", bufs=2, space="PSUM") as dps:
                # ---- WA: qkv + conv + chunk-0-head-0 dots ----
                with tc.tile_pool(name="qps", bufs=2, space="PSUM") as qps, \
                     tc.tile_pool(name="vps", bufs=1, space="PSUM") as vps:
                    cps_ref = [dps]
                    spe_head(0)                      # ACT: pooling gelu h0
                    for c in range(5):               # k blocks for head 0
                        qkv_block(qps, 8, c)
                        qkv_block(qps, 9, c)
                    qkv_block(qps, 0, 0)
                    qkv_block(qps, 1, 0)
                    pv = vps.tile([128, NJ * HPC], F32, name="pv")
                    for jb in range(NJ):
                        for t in range(4):
                            nc.tensor.matmul(
                                pv[:, jb * HPC:(jb + 1) * HPC],
                                xqv[:, 2 * t:2 * t + 2,
                                    jb * 128:(jb + 1) * 128],
                                wvv[:, 2 * t:2 * t + 2, :],
                                start=(t == 0), stop=(t == 3), perf_mode=DRM)
                    nc.vector.tensor_tensor(vcol[:], pv[:], vb72, ADD)
                    # conv (ACT gelus grouped) interleaved with k/q chains
                    fill = []
                    for blk in (10, 11, 12, 13):
                        for c in range(5):
                            fill.append((blk, c))
                    for blk in range(2, 8):
                        fill.append((blk, 0))
                    for blk in range(0, 8):
                        fill.append((blk, 1))
                    for blk in (14, 15):
                        for c in range(5):
                            fill.append((blk, c))
                    for c in range(2, 5):
                        for blk in range(0, 8):
                            fill.append((blk, c))
                    fi = 0
                    for h in range(HPC):
                        if h:
                            spe_head(h)
                        for bt in range(6):
                            conv_batch(dps, h, bt)
                            for _ in range(2):
                                if fi < len(fill):
                                    qkv_block(qps, *fill[fi])
                                    fi += 1
                    pt_tiles[(0, 0)] = []
                    for p in range(0, 9):            # exps start here
                        pd_batch(dps, 0, 0, p)
                        for _ in range(2):
                            if fi < len(fill):
                                qkv_block(qps, *fill[fi])
                                fi += 1
                    while fi < len(fill):
                        qkv_block(qps, *fill[fi])
                        fi += 1
                    for h in range(HPC):             # v' folds (not in place)
                        for jb in range(NJ):
                            nc.vector.scalar_tensor_tensor(
                                vpT[:, (h * NJ + jb) * 128:
                                    (h * NJ + jb + 1) * 128],
                                spe_bc[:, h * 128:(h + 1) * 128],
                                vcol[:, jb * HPC + h:jb * HPC + h + 1],
                                cbrS[:, (h * NJ + jb) * 128:
                                     (h * NJ + jb + 1) * 128], MULT, ADD)

                # ---- WB: attention pipeline + nn1 ----
                with tc.tile_pool(name="pms", bufs=1, space="PSUM") as pms, \
                     tc.tile_pool(name="pos", bufs=1, space="PSUM") as pos, \
                     tc.tile_pool(name="fps", bufs=2, space="PSUM") as fps:
                    units = [(c, h) for c in range(5) for h in range(HPC)]
                    pdlist = [(c, h, p) for (c, h) in units[1:]
                              for p in range(9)]
                    pi = 0

                    def emit_pd(n):
                        nonlocal pi
                        for _ in range(n):
                            if pi >= len(pdlist):
                                return
                            c, h, p = pdlist[pi]
                            if p == 0:
                                pt_tiles[(c, h)] = []
                            pd_batch(dps, c, h, p)
                            pi += 1

                    nnq = []

                    def emit_nn1(n):
                        for _ in range(n):
                            if nnq:
                                nn1_ebo(fps, *nnq.pop(0))

                    emit_pd(9)
                    for ui, (c, h) in enumerate(units):
                        pm = pms.tile([1, 512], F32, tag="m", name="pm")
                        po = pos.tile([128, 512], F32, tag="o", name="po")
                        for p in range(9):
                            pmpo_batch(pm, po, c, h, p)
                            emit_pd(1)
                            emit_nn1(1)
                        finish_head(pm, po, c, h)
                        if h == 3:
                            nnq.extend((c, e) for e in range(8))
                    emit_nn1(99)

                    if use_cc:
                        groups = [[0, 1], [2, 3], [4, 5], [6, 7]]
                        nc.gpsimd.collective_compute(
                            "ReduceScatter", ADD, replica_groups=groups,
                            ins=[part[0:CH, :].opt()], outs=[rs0[:].opt()])
                        nc.sync.dma_start(out_d[0:CH // 2, :], rs0[:])
                        nc.gpsimd.collective_compute(
                            "ReduceScatter", ADD, replica_groups=groups,
                            ins=[part[CH:D, :].opt()], outs=[rs1[:].opt()])
                        nc.sync.dma_start(out_d[CH // 2:CH, :], rs1[:])

    nc.compile()
    return nc


def _host_inputs(core, inp):
    b, half = core // 2, core % 2
    h0 = half * HPC
    x = np.asarray(inp["x"][b], dtype=np.float32)            # (D, N)
    Wqkv = np.asarray(inp["Wqkv"], dtype=np.float32)
    bqkv = np.asarray(inp["bqkv"], dtype=np.float32)
    Wspe = np.asarray(inp["Wspe"], dtype=np.float32)[:, :, 0, 0]   # (D, H)
    Wlocal = np.asarray(inp["Wlocal"], dtype=np.float32)     # (D, 8, 3, 3)
    Wnn1 = np.asarray(inp["Wnn1"], dtype=np.float32)
    bnn1 = np.asarray(inp["bnn1"], dtype=np.float32)

    chs = slice(h0 * HD, (h0 + HPC) * HD)

    def fold_bn(g, bta, mu, var):
        s = np.asarray(g, np.float64) / np.sqrt(np.asarray(var, np.float64) + EPS)
        return (s.astype(np.float32),
                (np.asarray(bta, np.float64) - np.asarray(mu, np.float64) * s)
                .astype(np.float32))


    # qkv moving operand: [p, dt, n]
    xq = np.ascontiguousarray(x.reshape(8, 128, N).transpose(1, 0, 2)
                              .reshape(128, 8 * N)).astype(E4)

    # conv image: raw reinterpret of x^T as (D, 48, 48); 3 dx-shifted
    # variants with 48-zero top/bottom pads, flattened per head-block
    ximg = np.ascontiguousarray(x.T).reshape(D, N)[chs]      # (512, 2304)
    xvar = np.zeros((128, HPC * 3 * PLANE + 128), np.float32)
    xv4 = xvar[:, :HPC * 3 * PLANE].reshape(128, HPC, 3, PLANE)
    img4 = ximg.reshape(HPC, 128, PS, PS)
    for hb in range(HPC):
        for vi, dx in enumerate((-1, 0, 1)):
            sh = np.zeros((128, PS, PS), np.float32)
            if dx == -1:
                sh[:, :, 1:] = img4[hb, :, :, :-1]
            elif dx == 1:
                sh[:, :, :-1] = img4[hb, :, :, 1:]
            else:
                sh = img4[hb]
            xv4[:, hb, vi, 48:48 + N] = sh.reshape(128, N)
    xvar[:, HPC * 3 * PLANE:] = 1.0 / 32.0          # conv bias plane
    xvar = np.ascontiguousarray(xvar).astype(E4)

    # q/k weights: 16 eb64 blocks (q0..q3,k0..k3 x dhalf), x8 prescale
    rows = np.concatenate(
        [np.arange(h0 * HD, (h0 + HPC) * HD) + s * D for s in range(2)])
    W8 = Wqkv[rows, :] * QS                                   # (1024, 1024)
    # wq[p, (t*16+blk)*128 + i*64 + m] = W8[blk*64+m, (2t+i)*128+p]
    wq = W8.reshape(16, 64, 4, 2, 128).transpose(4, 2, 0, 3, 1)
    wq = np.ascontiguousarray(wq.reshape(128, 64 * 128)).astype(E4)
    bqk64 = np.ascontiguousarray((QS * bqkv[rows]).reshape(16, 64).T
                                 .astype(np.float32))

    # v-sum weights [p, t, h]; bias pre-tiled [p, (jb, h)]
    vrows = np.arange(h0 * HD, (h0 + HPC) * HD) + 2 * D
    wvs = Wqkv[vrows, :].reshape(HPC, 128, D).sum(axis=1)     # (HPC, 1024)
    wvh = wvs.T.reshape(8, 128, HPC).transpose(1, 0, 2)
    wv = np.ascontiguousarray(wvh.reshape(128, 8 * HPC)).astype(E4)
    vb = bqkv[vrows].reshape(HPC, 128).sum(axis=1)
    vb72 = np.ascontiguousarray(
        np.tile(vb[None, None, :], (128, NJ, 1)).reshape(128, NJ * HPC)
        .astype(np.float32))

    # dense per-head conv weights, DoubleRow pair groups, x4 prescale
    wconv = np.zeros((HPC, 9, 128, 128), np.float32)
    for h in range(HPC):
        for co in range(128):
            g = co // 8
            cg = np.arange(g * 8, g * 8 + 8)
            for oi, (dy, dx) in enumerate(OFFS):
                wconv[h, oi, cg, co] = Wlocal[(h0 + h) * HD + co, :, dy + 1, dx + 1]
    # bn scale folded into weights; bias via the 1/32 plane (x32 here)
    bnc_s_full, bnc_b_full = fold_bn(inp["bnc_gamma"], inp["bnc_beta"],
                                     inp["bnc_mean"], inp["bnc_var"])
    sc = bnc_s_full[chs].reshape(HPC, 128)            # per (h, c)
    bc = bnc_b_full[chs].reshape(HPC, 128)
    wcv = np.zeros((128, HPC, 5, 2, 128), np.float32)
    for h in range(HPC):
        for g, (o1, o2) in enumerate(CVGROUPS):
            wcv[:, h, g, 0, :] = CVS * wconv[h, o1] * sc[h][None, :]
            if g < 4:
                wcv[:, h, g, 1, :] = CVS * wconv[h, o2] * sc[h][None, :]
            else:
                # encode 32*CVS*bc over 4 rows with residual refinement so
                # the (systematic) bias survives fp8 quantization
                S = 32.0 * CVS * bc[h]
                acc = np.zeros_like(S)
                for r in range(4):
                    step = np.asarray((S - acc) if r else S / 2, np.float32)
                    q = step.astype(E4).astype(np.float32)
                    wcv[r, h, g, 1, :] = q
                    acc += q
    wcv = np.ascontiguousarray(wcv.reshape(128, HPC * 5 * 256)).astype(E4)

    # spe block-diag matrix (folds 1/N pooling mean and attention scale)
    wspe = np.zeros((HPC, 128, 128), np.float32)
    for h in range(HPC):
        for gg in range(16):
            g = (h0 + h) * 16 + gg
            blk = Wspe[g * 8:(g + 1) * 8, :8]
            wspe[h, gg * 8:gg * 8 + 8, gg * 8:gg * 8 + 8] = blk.T
    wspe = np.ascontiguousarray(
        (wspe * (SCALE / N)).transpose(1, 0, 2).reshape(128, HPC * 128))

    bn_s, bn_b = fold_bn(inp["bn_gamma"], inp["bn_beta"], inp["bn_mean"],
                         inp["bn_var"])
    bnc_s, bnc_b = fold_bn(inp["bnc_gamma"], inp["bnc_beta"], inp["bnc_mean"],
                           inp["bnc_var"])
    shp = lambda a: np.ascontiguousarray(a[chs].reshape(HPC, 128).T)

    wnn1T = Wnn1[:, chs].T                                    # (512, 1024)
    wnn = np.ascontiguousarray(
        wnn1T.reshape(HPC, 128, D).transpose(1, 0, 2).reshape(128, HPC * D)
    ).astype(BF)
    bnn1h = np.ascontiguousarray((0.5 * bnn1).reshape(8, 128).T)

    cf32 = np.concatenate(
        [shp(bn_s), shp(bn_b), vb72, wspe, shp(bnc_s / CVS), shp(bnc_b)],
        axis=1).astype(np.float32)
    c8 = np.concatenate([wv, _ones_pair()], axis=1).astype(E4)
    return {
        "xq": xq, "xvar": xvar, "wq": wq, "c8": np.ascontiguousarray(c8),
        "bqk64": bqk64, "cf32": np.ascontiguousarray(cf32),
        "wcv": wcv,
        "wnn": wnn, "bnn1h": bnn1h,
    }


def _ones_pair():
    o = np.zeros((128, 32), np.float32)
    o[:, 0] = 1.0
    o[:, 16] = 1.0
    return o.astype(E4)


_NC = None


def kernel(**inputs):
    global _NC
    if _NC is None:
        _NC = _build()
    in_maps = [_host_inputs(c, inputs) for c in range(8)]
    res = run_bass_kernel_spmd(_NC, in_maps, core_ids=list(range(8)))
    out = np.empty((B, N, D), np.float32)
    for b in range(B):
        if USE_COLLECTIVE:
            ev, od = res.results[2 * b]["out"], res.results[2 * b + 1]["out"]
            t = np.empty((D, N), np.float32)
            t[0:256] = ev[0:256]
            t[256:512] = od[0:256]
            t[512:768] = ev[256:512]
            t[768:1024] = od[256:512]
        else:
            t = res.results[2 * b]["out"] + res.results[2 * b + 1]["out"]
        out[b] = t.T
    return out


def run_timed(**inputs):
    """Re-run with NTFF tracing to get HW exec time (best effort)."""
    global _NC
    if _NC is None:
        _NC = _build()
    in_maps = [_host_inputs(c, inputs) for c in range(8)]
    try:
        return run_bass_kernel_spmd(_NC, in_maps, core_ids=list(range(8)), trace=True)
    except Exception as e:
        print(f"trace run failed: {e}")
        return None
